# revision 47
# baseline (speedup 1.0000x reference)
"""Mixtral decoder layer (attention + top-2 MoE) on 8 TRN2 NeuronCores.

Self-contained: hardcodes all shapes/sharding. Strategy:
  - token-parallel attention (core c owns tokens [256c, 256c+256))
  - bf16 for all heavy matmuls/collectives, f32 residual + routing path
  - KV AllGather (2D-shaped, bf16) overlapped with Q projection + RoPE +
    diagonal-attention pass (local KV, partials to SBUF)
  - expert-parallel MoE (core c owns expert c), token compaction via
    matmul prefix-sums + indirect DMA scatter/gather, capacity 640
  - w AllGather before h AllGather; selection overlaps the h AllGather
  - MoE FFN with [128,512] weight DMAs, 2-deep mb prefetch
  - column-chunked ReduceScatter overlapped with second half of w2
"""

import os
from contextlib import ExitStack

import numpy as np
import ml_dtypes

KDBG = os.environ.get("KDBG", "0") == "1"

import concourse.mybir as mybir
import concourse.tile as tile
from concourse import bacc
from concourse.bass import IndirectOffsetOnAxis, ts
from concourse.bass_utils import run_bass_kernel_spmd

# ---- problem constants (hardcoded per contract) ----
T = 2048
HID = 2048
N_HEADS = 16
N_KV = 4
HD = 128  # head dim
QS = N_HEADS * HD  # 2048
KVS = N_KV * HD  # 512
FFN = 4096
NE = 8
EPS = 1e-5
ROPE_THETA = 10000.0
NC = 8  # cores
TS = T // NC  # 256 tokens per core
CAP = 640  # expert token capacity (mean 512, observed max ~561)
NEG = -1.0e30
SCALE = HD ** -0.5
H2 = HD // 2
NSPL = 2
NW = CAP // NSPL  # 320
NG = CAP // 128  # 5

F32R = mybir.dt.float32r
F32 = mybir.dt.float32
BF16 = mybir.dt.bfloat16
I32 = mybir.dt.int32

_cache = {}


def build():
    nc = bacc.Bacc("TRN2", num_devices=NC, debug=False)

    # ---------------- I/O ----------------
    x_in = nc.dram_tensor("x", [TS, HID], F32, kind="ExternalInput")
    cos_in = nc.dram_tensor("cos_t", [HD, TS], BF16, kind="ExternalInput")
    sin_in = nc.dram_tensor("sin_t", [HD, TS], BF16, kind="ExternalInput")
    wqk_in = nc.dram_tensor("wqkT", [HID, QS + KVS], BF16, kind="ExternalInput")
    wv_in = nc.dram_tensor("wvT", [HID, KVS], BF16, kind="ExternalInput")
    wo_in = nc.dram_tensor("woT", [QS, HID], BF16, kind="ExternalInput")
    gate_in = nc.dram_tensor("gateT", [HID, NE], F32R, kind="ExternalInput")
    w1_in = nc.dram_tensor("w1T", [HID, FFN], BF16, kind="ExternalInput")
    w3_in = nc.dram_tensor("w3T", [HID, FFN], BF16, kind="ExternalInput")
    w2_in = nc.dram_tensor("w2T", [FFN, HID], BF16, kind="ExternalInput")
    triu_in = nc.dram_tensor("triu128", [128, 128], F32, kind="ExternalInput")
    su16_in = nc.dram_tensor("su16", [16, 16], F32, kind="ExternalInput")
    id16_in = nc.dram_tensor("id16", [16, 16], F32, kind="ExternalInput")
    id128_in = nc.dram_tensor("id128", [128, 128], F32R, kind="ExternalInput")
    id128b_in = nc.dram_tensor("id128b", [128, 128], BF16, kind="ExternalInput")
    md0_in = nc.dram_tensor("md0", [128, 256], BF16, kind="ExternalInput")
    md1_in = nc.dram_tensor("md1", [128, 256], BF16, kind="ExternalInput")
    bias_in = nc.dram_tensor("bias_c", [128, 16], F32, kind="ExternalInput")
    esel_in = nc.dram_tensor("e_sel", [1, NE], F32, kind="ExternalInput")
    iota_in = nc.dram_tensor("iota_c", [128, 16], I32, kind="ExternalInput")

    y_out = nc.dram_tensor("y", [TS, HID], F32, kind="ExternalOutput")
    if KDBG:
        dbg_q = nc.dram_tensor("dbg_q", [16 * 128, 256], BF16,
                               kind="ExternalOutput")
        dbg_k = nc.dram_tensor("dbg_k", [4 * 128, 256], BF16,
                               kind="ExternalOutput")
        dbg_v = nc.dram_tensor("dbg_v", [2 * 128, KVS], BF16,
                               kind="ExternalOutput")
        dbg_att = nc.dram_tensor("dbg_att", [16 * 128, 256], BF16,
                                 kind="ExternalOutput")
        dbg_r2 = nc.dram_tensor("dbg_r2", [TS, HID], F32,
                                kind="ExternalOutput")
        dbg_h2 = nc.dram_tensor("dbg_h2", [TS, HID], F32,
                                kind="ExternalOutput")
        dbg_w = nc.dram_tensor("dbg_w", [TS, NE], F32, kind="ExternalOutput")
        dbg_idx = nc.dram_tensor("dbg_idx", [NG * 128, 1], I32,
                                 kind="ExternalOutput")
        dbg_wg = nc.dram_tensor("dbg_wg", [NG * 128, 1], F32,
                                kind="ExternalOutput")
        dbg_xg = nc.dram_tensor("dbg_xg", [16 * 128, CAP], BF16,
                                kind="ExternalOutput")
        dbg_g = nc.dram_tensor("dbg_g", [32 * 128, CAP], BF16,
                               kind="ExternalOutput")
        dbg_or = nc.dram_tensor("dbg_or", [NG * 128, HID], BF16,
                                kind="ExternalOutput")
        dbg_katt = nc.dram_tensor("dbg_katt", [4 * 128, 256], BF16,
                                  kind="ExternalOutput")
        dbg_vatt = nc.dram_tensor("dbg_vatt", [2 * 128, KVS], BF16,
                                  kind="ExternalOutput")
        dbg_kvco = nc.dram_tensor("dbg_kvco", [128, 2048], BF16,
                                  kind="ExternalOutput")
        dbg_pvb = nc.dram_tensor("dbg_pvb", [16 * 128, 256], F32,
                                 kind="ExternalOutput")
        dbg_rsb = nc.dram_tensor("dbg_rsb", [16, 256], F32,
                                 kind="ExternalOutput")

    # ---------------- internal DRAM (collectives) ----------------
    # kv block per core: rows 0..63 = K (4 kv groups x 16 rows, each [HD,TS]
    # flattened), rows 64..127 = V ([TS, KVS] flattened). 2D shape so the
    # collective parallelizes across partition rows.
    HW = HID + NE  # h row + routing weights fused into one AllGather
    kv_ci = nc.dram_tensor("kv_ci", [128, 2048], BF16)
    kv_co = nc.dram_tensor("kv_co", [NC * 128, 2048], BF16, addr_space="Shared")
    h_ci = nc.dram_tensor("h_ci", [TS, HW], BF16)
    h_co = nc.dram_tensor("h_co", [T, HW], BF16, addr_space="Shared")
    moe_q = [nc.dram_tensor(f"moe_q{i}", [T, 512], BF16) for i in range(4)]
    rs_q = [nc.dram_tensor(f"rs_q{i}", [TS, 512], BF16) for i in range(4)]
    idx_buf = nc.dram_tensor("idx_buf", [T, 1], I32)
    wcol_d = nc.dram_tensor("wcol_d", [T, 1], F32)

    RG = [list(range(NC))]

    with tile.TileContext(nc, pool_alloc_mode="queue") as tc, \
         ExitStack() as gctx:
        const = gctx.enter_context(tc.tile_pool(name="const", bufs=1))
        np_pool = gctx.enter_context(tc.tile_pool(name="np_pool", bufs=1))
        r2_pool = gctx.enter_context(tc.tile_pool(name="r2_pool", bufs=1))
        xpool = gctx.enter_context(tc.tile_pool(name="xpool", bufs=1))

        # x first on the sync queue so norm1 can start ASAP
        x_tiles = []
        for j in range(2):
            xt = xpool.tile([128, HID], F32, name=f"x_{j}")
            nc.sync.dma_start(xt[:], x_in[ts(j, 128), :])
            x_tiles.append(xt)

        _cq = [0]

        def cdma(name, shape, dt, src):
            t = const.tile(shape, dt, name=name)
            q = nc.sync if _cq[0] % 2 == 0 else nc.scalar
            _cq[0] += 1
            q.dma_start(t[:], src[:])
            return t

        su16 = cdma("su16s", [16, 16], F32, su16_in)
        id16 = cdma("id16s", [16, 16], F32, id16_in)
        id128 = cdma("id128s", [128, 128], F32R, id128_in)
        id128b = cdma("id128bs", [128, 128], BF16, id128b_in)
        md0 = cdma("md0s", [128, 256], BF16, md0_in)
        md1 = cdma("md1s", [128, 256], BF16, md1_in)
        bias_c = cdma("bias_cs", [128, 16], F32, bias_in)
        cosb = cdma("cosbs", [HD, TS], BF16, cos_in)
        sinb = cdma("sinbs", [HD, TS], BF16, sin_in)
        iota_sb = cdma("iota_sbs", [128, 16], I32, iota_in)
        esel = cdma("esels", [1, NE], F32, esel_in)
        triu_f = cdma("triu_fs", [128, 128], F32, triu_in)
        epsb = const.tile([128, 1], F32, name="epsb")
        nc.vector.memset(epsb[:], EPS)
        ones1_f = const.tile([1, 128], F32, name="ones1_f")
        nc.vector.memset(ones1_f[:], 1.0)
        onesp_f = const.tile([128, 1], F32, name="onesp_f")
        nc.vector.memset(onesp_f[:], 1.0)
        onesp_b = const.tile([128, 1], BF16, name="onesp_b")
        nc.vector.tensor_copy(onesp_b[:], onesp_f[:])

        def rms_norm(src_tiles, dst_pool, dst_name, dst_dt):
            out = []
            for j, xt in enumerate(src_tiles):
                scratch = np_pool.tile([128, HID], F32, name="nscratch",
                                       tag="nscratch")
                ssq = np_pool.tile([128, 1], F32, name="nssq", tag="nssq")
                nc.scalar.activation(
                    scratch[:], xt[:], mybir.ActivationFunctionType.Square,
                    accum_out=ssq[:])
                std = np_pool.tile([128, 1], F32, name="nstd", tag="nstd")
                nc.scalar.activation(
                    std[:], ssq[:], mybir.ActivationFunctionType.Sqrt,
                    bias=epsb[:], scale=1.0 / HID)
                rstd = np_pool.tile([128, 1], F32, name="nrstd", tag="nrstd")
                nc.vector.reciprocal(rstd[:], std[:])
                hn = dst_pool.tile([128, HID], dst_dt, name=f"{dst_name}_{j}")
                nc.vector.tensor_scalar_mul(hn[:], xt[:], rstd[:])
                out.append(hn)
            return out

        # ---- idx sentinel early (moe zeroing deferred to attention) ----
        with tc.tile_pool(name="zpool", bufs=1) as zpool:
            zidx = zpool.tile([128, 16], I32, name="zidx")
            nc.vector.memset(zidx[:], 4095)
            nc.gpsimd.dma_start(
                idx_buf[:].rearrange("(j p) one -> p (j one)", p=128),
                zidx[:])

        # ================= phase 1: norm1, X^T =================
        actx = ExitStack()  # pools that live through attention/o_proj
        qkT_pool = actx.enter_context(tc.tile_pool(name="qkT_pool", bufs=1))
        v_pool = actx.enter_context(tc.tile_pool(name="v_pool", bufs=1))
        att_pool = actx.enter_context(tc.tile_pool(name="att_pool", bufs=1))
        pvb_pool = actx.enter_context(tc.tile_pool(name="pvb_pool", bufs=1))

        kT = [None] * N_KV
        qT = [None] * N_HEADS
        v_tiles = []

        with tc.tile_pool(name="hn_pool", bufs=1) as hn_pool, \
             tc.tile_pool(name="xt_pool", bufs=1) as xt_pool, \
             tc.tile_pool(name="wv_pool", bufs=1) as wv_pool, \
             tc.tile_pool(name="wqk_pool", bufs=8) as wqk_pool, \
             tc.tile_pool(name="rope_pool", bufs=4) as rope_pool, \
             tc.tile_pool(name="ps_tp", bufs=2, space="PSUM") as ps_tp, \
             tc.tile_pool(name="ps_mm", bufs=4, space="PSUM") as ps_mm, \
             tc.tile_pool(name="psv", bufs=2, space="PSUM") as psv:
            # prefetch V weights on the gpsimd queue (used after K block)
            wv_tiles = []
            for k in range(16):
                wvt = wv_pool.tile([128, KVS], BF16, name=f"wv_{k}")
                nc.gpsimd.dma_start(wvt[:], wv_in[ts(k, 128), :])
                wv_tiles.append(wvt)

            hn_tiles = rms_norm(x_tiles, hn_pool, "hn", BF16)

            xT = []
            for k in range(16):
                xtile = xt_pool.tile([128, 256], BF16, name=f"xT_{k}")
                for j in range(2):
                    tp = ps_tp.tile([128, 128], BF16, name="tp_ps", tag="tp",
                                    space="PSUM")
                    nc.tensor.transpose(tp[:], hn_tiles[j][:, ts(k, 128)],
                                        id128b[:])
                    nc.vector.tensor_copy(xtile[:, ts(j, 128)], tp[:])
                xT.append(xtile)

            def rope(src):
                rot = rope_pool.tile([128, 256], BF16, name="rrot", tag="rot")
                nc.sync.dma_start(rot[0:H2, :], src[H2:HD, :])
                nc.sync.dma_start(rot[H2:HD, :], src[0:H2, :])
                ta = rope_pool.tile([128, 256], BF16, name="rta", tag="ra")
                tb = rope_pool.tile([128, 256], BF16, name="rtb", tag="rb")
                nc.vector.tensor_mul(ta[:], src[:], cosb[:])
                nc.vector.tensor_mul(tb[:], rot[:], sinb[:])
                return ta, tb

            def proj_block(ob, names):
                # one 512-col output block of wqkT -> 4 [128,256] bf16 tiles
                pss = [ps_mm.tile([128, 256], F32, name="qk_ps", tag="mm",
                                  space="PSUM") for _ in range(4)]
                for k in range(16):
                    q = nc.sync if (k % 2 == 0) else nc.scalar
                    wt = wqk_pool.tile([128, 512], BF16, name="wqk_t", tag="w")
                    q.dma_start(wt[:], wqk_in[ts(k, 128), ts(ob, 512)])
                    for oi in range(4):
                        nc.tensor.matmul(pss[oi][:], wt[:, ts(oi, 128)],
                                         xT[k][:], start=(k == 0),
                                         stop=(k == 15))
                outs = []
                for oi in range(4):
                    dst = qkT_pool.tile([128, 256], BF16, name=names[oi])
                    nc.vector.tensor_copy(dst[:], pss[oi][:])
                    ta, tb = rope(dst)
                    nc.vector.tensor_add(dst[:], ta[:], tb[:])
                    outs.append(dst)
                return outs

            # --- K first (output cols 2048..2560) ---
            kT[0:4] = proj_block(4, [f"kT_{i}" for i in range(4)])

            # --- V ---
            vps = [psv.tile([128, KVS], F32, name="v_ps", tag="v",
                            space="PSUM") for _ in range(2)]
            for k in range(16):
                for j in range(2):
                    nc.tensor.matmul(vps[j][:], xT[k][:, ts(j, 128)],
                                     wv_tiles[k][:], start=(k == 0),
                                     stop=(k == 15))
            for j in range(2):
                vt = v_pool.tile([128, KVS], BF16, name=f"v_{j}")
                nc.vector.tensor_copy(vt[:], vps[j][:])
                v_tiles.append(vt)

            # --- stage K/V and kick the KV AllGather ---
            for kv in range(N_KV):
                nc.sync.dma_start(
                    kv_ci[kv * 16:(kv + 1) * 16, :].rearrange(
                        "a (b t) -> (a b) t", t=TS),
                    kT[kv][:])
            for j in range(2):
                nc.sync.dma_start(
                    kv_ci[64 + 32 * j:64 + 32 * (j + 1), :].rearrange(
                        "a (b d) -> (a b) d", d=KVS),
                    v_tiles[j][:])
            nc.gpsimd.collective_compute(
                "AllGather", mybir.AluOpType.bypass, replica_groups=RG,
                ins=[kv_ci[:]], outs=[kv_co[:]])

            # --- Q (overlaps the AllGather) ---
            for ob in range(4):
                qT[4 * ob:4 * ob + 4] = proj_block(
                    ob, [f"qT_{4 * ob + i}" for i in range(4)])

            if KDBG:
                for o in range(16):
                    nc.sync.dma_start(dbg_q[ts(o, 128), :], qT[o][:])
                for kv in range(N_KV):
                    nc.sync.dma_start(dbg_k[ts(kv, 128), :], kT[kv][:])
                for j in range(2):
                    nc.sync.dma_start(dbg_v[ts(j, 128), :], v_tiles[j][:])

        # ============ pass B: diagonal attention with local KV ============
        pvB = [None] * N_HEADS
        rsB = [None] * N_HEADS
        with tc.tile_pool(name="eb_pool", bufs=4) as eb_pool, \
             tc.tile_pool(name="ps_bs", bufs=2, space="PSUM") as ps_bs, \
             tc.tile_pool(name="ps_bpv", bufs=2, space="PSUM") as ps_bpv, \
             tc.tile_pool(name="ps_brs", bufs=2, space="PSUM") as ps_brs:
            for kv in range(N_KV):
                for hp in range(2):
                    heads = [4 * kv + 2 * hp, 4 * kv + 2 * hp + 1]
                    pv_ps = ps_bpv.tile([128, 512], F32, name="bpv_ps",
                                        tag="pv", space="PSUM")
                    rs_ps = ps_brs.tile([1, 512], F32, name="brs_ps",
                                        tag="rs", space="PSUM")
                    for half in range(2):
                        sps = ps_bs.tile([128, 512], F32, name="bs_ps",
                                         tag="s", space="PSUM")
                        for i in range(2):
                            nc.tensor.matmul(sps[:, ts(i, 256)],
                                             kT[kv][:, ts(half, 128)],
                                             qT[heads[i]][:],
                                             start=True, stop=True)
                        et = eb_pool.tile([128, 512], BF16, name="bet",
                                          tag="e")
                        nc.scalar.activation(
                            et[:], sps[:],
                            mybir.ActivationFunctionType.Exp, scale=SCALE)
                        msk = md0 if half == 0 else md1
                        for i in range(2):
                            nc.vector.tensor_mul(et[:, ts(i, 256)],
                                                 et[:, ts(i, 256)], msk[:])
                        nc.tensor.matmul(pv_ps[:],
                                         v_tiles[half][:, ts(kv, 128)],
                                         et[:], start=(half == 0),
                                         stop=(half == 1))
                        nc.tensor.matmul(rs_ps[:], onesp_b[:], et[:],
                                         start=(half == 0),
                                         stop=(half == 1))
                    for i in range(2):
                        pb = pvb_pool.tile([128, 256], F32,
                                           name=f"pvB_{heads[i]}")
                        nc.vector.tensor_copy(pb[:], pv_ps[:, ts(i, 256)])
                        pvB[heads[i]] = pb
                        rb = pvb_pool.tile([1, 256], F32,
                                           name=f"rsB_{heads[i]}")
                        nc.vector.tensor_copy(rb[:], rs_ps[:, ts(i, 256)])
                        rsB[heads[i]] = rb
                        if KDBG:
                            nc.sync.dma_start(dbg_pvb[ts(heads[i], 128), :],
                                              pb[:])
                            nc.sync.dma_start(
                                dbg_rsb[heads[i]:heads[i] + 1, :], rb[:])

        # ================= pass A: gathered attention =================
        attnT = [None] * N_HEADS
        with tc.tile_pool(name="kvt_pool", bufs=1) as kvt_pool, \
             tc.tile_pool(name="e_pool", bufs=6) as e_pool, \
             tc.tile_pool(name="sc_pool", bufs=4) as sc_pool, \
             tc.tile_pool(name="ps_s", bufs=3, space="PSUM") as ps_s, \
             tc.tile_pool(name="ps_pv", bufs=2, space="PSUM") as ps_pv, \
             tc.tile_pool(name="ps_rs", bufs=2, space="PSUM") as ps_rs:
            katt = [[None] * N_KV for _ in range(NC)]
            vatt = [None] * 16
            qs = [nc.sync, nc.scalar, nc.gpsimd]
            qi = 0
            for r in range(NC):
                for kv in range(N_KV):
                    kt = kvt_pool.tile([128, 256], BF16, name=f"k_{r}_{kv}")
                    qs[qi % 3].dma_start(
                        kt[:],
                        kv_co[128 * r + 16 * kv:128 * r + 16 * (kv + 1), :]
                        .rearrange("a (b t) -> (a b) t", t=TS))
                    katt[r][kv] = kt
                    qi += 1
                for j in range(2):
                    vt = kvt_pool.tile([128, KVS], BF16, name=f"v_{r}_{j}")
                    qs[qi % 3].dma_start(
                        vt[:],
                        kv_co[128 * r + 64 + 32 * j:128 * r + 64 + 32 * (j + 1),
                              :].rearrange("a (b d) -> (a b) d", d=KVS))
                    vatt[2 * r + j] = vt
                    qi += 1

            # zero the moe scatter buffers (DMA engines idle during pass A)
            ztf = kvt_pool.tile([128, 512], F32, name="ztf")
            nc.vector.memset(ztf[:], 0.0)
            ztile = kvt_pool.tile([128, 512], BF16, name="ztile")
            nc.vector.tensor_copy(ztile[:], ztf[:])
            for q in range(4):
                for i in range(T // 128):
                    nc.gpsimd.dma_start(moe_q[q][ts(i, 128), :], ztile[:])

            if KDBG:
                for kv in range(N_KV):
                    nc.sync.dma_start(dbg_katt[ts(kv, 128), :],
                                      katt[0][kv][:])
                for j in range(2):
                    nc.sync.dma_start(dbg_vatt[ts(j, 128), :], vatt[j][:])
                kvraw = kvt_pool.tile([128, 2048], BF16, name="kvraw")
                nc.sync.dma_start(kvraw[:], kv_co[0:128, :])
                nc.sync.dma_start(dbg_kvco[:], kvraw[:])

            for kv in range(N_KV):
                for hp in range(2):
                    heads = [4 * kv + 2 * hp, 4 * kv + 2 * hp + 1]
                    pv_ps = ps_pv.tile([128, 512], F32, name="pv_ps",
                                       tag="pv", space="PSUM")
                    rs_ps = ps_rs.tile([1, 512], F32, name="rs_ps",
                                       tag="rs", space="PSUM")
                    for sg in range(16):
                        sps = ps_s.tile([128, 512], F32, name="s_ps", tag="s",
                                        space="PSUM")
                        for i in range(2):
                            nc.tensor.matmul(sps[:, ts(i, 256)],
                                             katt[sg // 2][kv][:,
                                                              ts(sg % 2, 128)],
                                             qT[heads[i]][:],
                                             start=True, stop=True)
                        et = e_pool.tile([128, 512], BF16, name="et", tag="e")
                        nc.scalar.activation(
                            et[:], sps[:],
                            mybir.ActivationFunctionType.Exp,
                            bias=bias_c[:, sg:sg + 1], scale=SCALE)
                        nc.tensor.matmul(pv_ps[:], vatt[sg][:, ts(kv, 128)],
                                         et[:], start=(sg == 0),
                                         stop=(sg == 15))
                        nc.tensor.matmul(rs_ps[:], onesp_b[:], et[:],
                                         start=(sg == 0), stop=(sg == 15))
                    for i in range(2):
                        h = heads[i]
                        rs_sb = sc_pool.tile([1, 256], F32R, name="rs_sb",
                                             tag="rsb")
                        nc.vector.tensor_add(rs_sb[:], rs_ps[:, ts(i, 256)],
                                             rsB[h][:])
                        with nc.allow_low_precision(
                                reason="f32r recip for PE bcast"):
                            nc.vector.reciprocal(rs_sb[:], rs_sb[:])
                        bc_ps = ps_s.tile([128, 256], F32, name="bc_ps",
                                          tag="s", space="PSUM")
                        nc.tensor.matmul(bc_ps[:], ones1_f[:].bitcast(F32R),
                                         rs_sb[:], start=True, stop=True)
                        bc_sb = sc_pool.tile([128, 256], F32, name="bc_sb",
                                             tag="bcs")
                        nc.scalar.copy(bc_sb[:], bc_ps[:])
                        pvt = sc_pool.tile([128, 256], F32, name="pv_tot",
                                           tag="pvt")
                        nc.vector.tensor_add(pvt[:], pv_ps[:, ts(i, 256)],
                                             pvB[h][:])
                        at = att_pool.tile([128, 256], BF16,
                                           name=f"attnT_{h}")
                        nc.vector.tensor_mul(at[:], pvt[:], bc_sb[:])
                        attnT[h] = at

        # ============ o_proj + residual ============
        resid2 = []
        with tc.tile_pool(name="wo_pool", bufs=5) as wo_pool, \
             tc.tile_pool(name="ps5", bufs=8, space="PSUM") as ps5:
            o_ps = [[ps5.tile([128, 512], F32, name="o_ps", tag="t",
                              space="PSUM") for _ in range(4)]
                    for _ in range(2)]
            for k in range(16):
                q = nc.sync if (k % 2 == 0) else nc.scalar
                wt = wo_pool.tile([128, HID], BF16, name="wo_t", tag="w")
                q.dma_start(wt[:], wo_in[ts(k, 128), :])
                for j in range(2):
                    for nb in range(4):
                        nc.tensor.matmul(o_ps[j][nb][:],
                                         attnT[k][:, ts(j, 128)],
                                         wt[:, ts(nb, 512)], start=(k == 0),
                                         stop=(k == 15))
            for j in range(2):
                r2 = r2_pool.tile([128, HID], F32, name=f"resid2_{j}")
                for nb in range(4):
                    nc.vector.tensor_add(r2[:, ts(nb, 512)], o_ps[j][nb][:],
                                         x_tiles[j][:, ts(nb, 512)])
                resid2.append(r2)
        if KDBG:
            for h in range(16):
                nc.sync.dma_start(dbg_att[ts(h, 128), :], attnT[h][:])
            for j in range(2):
                nc.sync.dma_start(dbg_r2[ts(j, 128), :], resid2[j][:])
        actx.close()

        # ============ norm2 + gate + fused h/w AllGather ============
        with tc.tile_pool(name="h2_pool", bufs=1) as h2_pool:
            h2n_tiles = rms_norm(resid2, h2_pool, "h2n", F32R)
            wful_tiles = []

            with tc.tile_pool(name="x2t_pool", bufs=1) as x2t_pool, \
                 tc.tile_pool(name="gate_pool", bufs=2) as gate_pool, \
                 tc.tile_pool(name="ps6t", bufs=2, space="PSUM") as ps6t, \
                 tc.tile_pool(name="ps6b", bufs=2, space="PSUM") as ps6b:
                x2T = []
                for k in range(16):
                    row = []
                    for j in range(2):
                        dst = x2t_pool.tile([128, 128], F32R,
                                            name=f"x2T_{k}_{j}")
                        tp = ps6t.tile([128, 128], F32R, name="tp2_ps",
                                       tag="t", space="PSUM")
                        nc.tensor.transpose(tp[:],
                                            h2n_tiles[j][:, ts(k, 128)],
                                            id128[:])
                        nc.vector.tensor_copy(dst[:], tp[:])
                        row.append(dst)
                    x2T.append(row)

                gsb = gate_pool.tile([128, 16 * NE], F32R, name="gsb")
                nc.sync.dma_start(
                    gsb[:].rearrange("p (k e) -> p k e", e=NE),
                    gate_in[:].rearrange("(k p) e -> p k e", p=128))
                for j in range(2):
                    gps = ps6b.tile([128, NE], F32, name="g_ps", tag="t",
                                    space="PSUM")
                    for k in range(16):
                        nc.tensor.matmul(
                            gps[:], x2T[k][j][:],
                            gsb[:].rearrange("p (k e) -> p k e", e=NE)[:, k, :],
                            start=(k == 0), stop=(k == 15))
                    lg = gate_pool.tile([128, NE], F32, name="lg", tag="g1")
                    nc.vector.tensor_copy(lg[:], gps[:])
                    mx = gate_pool.tile([128, 1], F32, name="gmx", tag="g2")
                    nc.vector.reduce_max(mx[:], lg[:],
                                         axis=mybir.AxisListType.X)
                    nmx = gate_pool.tile([128, 1], F32, name="gnmx", tag="g3")
                    nc.vector.tensor_scalar_mul(nmx[:], mx[:], -1.0)
                    p = gate_pool.tile([128, NE], F32, name="gp", tag="g4")
                    nc.scalar.activation(p[:], lg[:],
                                         mybir.ActivationFunctionType.Exp,
                                         bias=nmx[:])
                    v1 = gate_pool.tile([128, 1], F32, name="gv1", tag="g5")
                    nc.vector.reduce_max(v1[:], p[:],
                                         axis=mybir.AxisListType.X)
                    ge1 = gate_pool.tile([128, NE], F32, name="gge1", tag="g6")
                    nc.vector.tensor_single_scalar(ge1[:], p[:], v1[:],
                                                   op=mybir.AluOpType.is_ge)
                    pt = gate_pool.tile([128, NE], F32, name="gpt", tag="g7")
                    nc.vector.tensor_mul(pt[:], p[:], ge1[:])
                    p2 = gate_pool.tile([128, NE], F32, name="gp2", tag="g8")
                    nc.vector.tensor_sub(p2[:], p[:], pt[:])
                    v2 = gate_pool.tile([128, 1], F32, name="gv2", tag="g9")
                    nc.vector.reduce_max(v2[:], p2[:],
                                         axis=mybir.AxisListType.X)
                    m2 = gate_pool.tile([128, NE], F32, name="gm2", tag="g10")
                    nc.vector.tensor_single_scalar(m2[:], p[:], v2[:],
                                                   op=mybir.AluOpType.is_ge)
                    pm = gate_pool.tile([128, NE], F32, name="gpm", tag="g11")
                    nc.vector.tensor_mul(pm[:], p[:], m2[:])
                    s12 = gate_pool.tile([128, 1], F32, name="gs12", tag="g12")
                    nc.vector.tensor_add(s12[:], v1[:], v2[:])
                    nc.vector.reciprocal(s12[:], s12[:])
                    wful = h2_pool.tile([128, NE], F32, name=f"wful_{j}")
                    nc.vector.tensor_scalar_mul(wful[:], pm[:], s12[:])
                    wful_tiles.append(wful)
                    if KDBG:
                        nc.sync.dma_start(dbg_w[ts(j, 128), :], wful[:])

            # h cast (+ fused routing cols) + stage + single AllGather
            with tc.tile_pool(name="h2b_pool", bufs=2) as h2b_pool:
                for j in range(2):
                    hb = h2b_pool.tile([128, HW], BF16, name="h2b", tag="b")
                    nc.vector.tensor_copy(hb[:, 0:HID], h2n_tiles[j][:])
                    nc.vector.tensor_copy(hb[:, HID:HW], wful_tiles[j][:])
                    nc.sync.dma_start(h_ci[ts(j, 128), :], hb[:])
                    if KDBG:
                        nc.sync.dma_start(dbg_h2[ts(j, 128), :],
                                          h2n_tiles[j][:].bitcast(F32))
            nc.gpsimd.collective_compute(
                "AllGather", mybir.AluOpType.bypass, replica_groups=RG,
                ins=[h_ci[:]], outs=[h_co[:]])

        # ====== FFN weight prefetch (overlaps AllGather + selection) ======
        gat_pool = gctx.enter_context(tc.tile_pool(name="gat_pool", bufs=1))
        g_pool = gctx.enter_context(tc.tile_pool(name="g_pool", bufs=1))
        fctx = ExitStack()
        w13_pool = fctx.enter_context(tc.tile_pool(name="w13_pool", bufs=40))

        def load_w13(mb):
            w1ts, w3ts = [], []
            for k in range(16):
                w1t = w13_pool.tile([128, 512], BF16, name="w1_t", tag="w1")
                nc.sync.dma_start(w1t[:], w1_in[ts(k, 128), ts(mb, 512)])
                w1ts.append(w1t)
                w3t = w13_pool.tile([128, 512], BF16, name="w3_t", tag="w3")
                nc.scalar.dma_start(w3t[:], w3_in[ts(k, 128), ts(mb, 512)])
                w3ts.append(w3t)
            return w1ts, w3ts

        w13_pre = {mb: load_w13(mb) for mb in range(2)}

        # ================= expert token selection =================
        with tc.tile_pool(name="sel_pool", bufs=1) as sel_pool, \
             tc.tile_pool(name="ps7", bufs=2, space="PSUM") as ps7:
            wall_b = sel_pool.tile([128, 16 * NE], BF16, name="wall_b")
            nc.sync.dma_start(
                wall_b[:].rearrange("p (k e) -> p k e", e=NE),
                h_co[:, HID:HW].rearrange("(k p) e -> p k e", p=128))
            wall = sel_pool.tile([128, 16 * NE], F32, name="wall")
            nc.vector.tensor_copy(wall[:], wall_b[:])
            eselb = sel_pool.tile([128, NE], F32, name="eselb")
            nc.gpsimd.partition_broadcast(eselb[:], esel[:])
            wsel = sel_pool.tile([128, 16 * NE], F32, name="wsel")
            nc.vector.tensor_tensor(
                wsel[:].rearrange("p (k e) -> p k e", e=NE),
                wall[:].rearrange("p (k e) -> p k e", e=NE),
                eselb[:].rearrange("p (o e) -> p o e", o=1)
                .to_broadcast([128, 16, NE]),
                op=mybir.AluOpType.mult)
            wcol = sel_pool.tile([128, 16], F32, name="wcol")
            nc.vector.reduce_sum(
                wcol[:], wsel[:].rearrange("p (k e) -> p k e", e=NE),
                axis=mybir.AxisListType.X)
            nc.sync.dma_start(
                wcol_d[:].rearrange("(j p) one -> p (j one)", p=128), wcol[:])
            mall = sel_pool.tile([128, 16], F32, name="mall")
            nc.vector.tensor_single_scalar(mall[:], wcol[:], 0.0,
                                           op=mybir.AluOpType.is_gt)
            rank_ps = ps7.tile([128, 16], F32, name="rank_ps", tag="a",
                               space="PSUM")
            nc.tensor.matmul(rank_ps[:], triu_f[:], mall[:], start=True,
                             stop=True)
            tot_ps = ps7.tile([1, 16], F32, name="tot_ps", tag="b",
                              space="PSUM")
            nc.tensor.matmul(tot_ps[:], onesp_f[:], mall[:], start=True,
                             stop=True)
            tot = sel_pool.tile([1, 16], F32, name="tot")
            nc.vector.tensor_copy(tot[:], tot_ps[:])
            totT_ps = ps7.tile([16, 1], F32, name="totT_ps", tag="b",
                               space="PSUM")
            nc.tensor.matmul(totT_ps[:], tot[:], ones1_f[:, 0:1], start=True,
                             stop=True)
            totT = sel_pool.tile([16, 1], F32, name="totT")
            nc.vector.tensor_copy(totT[:], totT_ps[:])
            ex_ps = ps7.tile([16, 1], F32, name="ex_ps", tag="b", space="PSUM")
            nc.tensor.matmul(ex_ps[:], su16[:], totT[:], start=True, stop=True)
            exT = sel_pool.tile([16, 1], F32, name="exT")
            nc.vector.tensor_copy(exT[:], ex_ps[:])
            exr_ps = ps7.tile([1, 16], F32, name="exr_ps", tag="b",
                              space="PSUM")
            nc.tensor.matmul(exr_ps[:], exT[:], id16[:], start=True, stop=True)
            exr = sel_pool.tile([1, 16], F32, name="exr")
            nc.vector.tensor_copy(exr[:], exr_ps[:])
            exb_ps = ps7.tile([128, 16], F32, name="exb_ps", tag="b",
                              space="PSUM")
            nc.tensor.matmul(exb_ps[:], ones1_f[:], exr[:], start=True,
                             stop=True)
            posf = sel_pool.tile([128, 16], F32, name="posf")
            nc.vector.tensor_copy(posf[:], rank_ps[:])
            nc.vector.tensor_add(posf[:], posf[:], exb_ps[:])
            adj = sel_pool.tile([128, 16], F32, name="adj")
            nc.vector.tensor_scalar(
                adj[:], mall[:], -4096.0, 4095.0,
                op0=mybir.AluOpType.mult, op1=mybir.AluOpType.add)
            nc.vector.tensor_add(posf[:], posf[:], adj[:])
            posi = sel_pool.tile([128, 16], I32, name="posi")
            nc.vector.tensor_copy(posi[:], posf[:])
            for j in range(16):
                nc.gpsimd.indirect_dma_start(
                    out=idx_buf[:],
                    out_offset=IndirectOffsetOnAxis(ap=posi[:, j:j + 1],
                                                    axis=0),
                    in_=iota_sb[:, j:j + 1],
                    in_offset=None,
                    bounds_check=T - 1, oob_is_err=False)

        idx_tiles, wg_tiles = [], []
        for g in range(NG):
            it = gat_pool.tile([128, 1], I32, name=f"idx_{g}")
            nc.sync.dma_start(it[:], idx_buf[ts(g, 128), :])
            idx_tiles.append(it)
            wg = gat_pool.tile([128, 1], F32, name=f"wg_{g}")
            nc.vector.memset(wg[:], 0.0)
            nc.gpsimd.indirect_dma_start(
                out=wg[:], out_offset=None,
                in_=wcol_d[:],
                in_offset=IndirectOffsetOnAxis(ap=it[:, 0:1], axis=0),
                bounds_check=T - 1, oob_is_err=False)
            wg_tiles.append(wg)
            if KDBG:
                nc.sync.dma_start(dbg_idx[ts(g, 128), :], it[:])
                nc.sync.dma_start(dbg_wg[ts(g, 128), :], wg[:])

        # ================= gather + expert FFN =================
        g_tiles = []

        with tc.tile_pool(name="xgt_pool", bufs=1) as xgt_pool:
            xgT = [xgt_pool.tile([128, CAP], BF16, name=f"xgT_{k}")
                   for k in range(16)]
            with tc.tile_pool(name="row_pool", bufs=2) as row_pool, \
                 tc.tile_pool(name="ps8", bufs=3, space="PSUM") as ps8:
                for g in range(NG):
                    rows = row_pool.tile([128, HW], BF16, name="xg_rows",
                                         tag="rows")
                    nc.gpsimd.indirect_dma_start(
                        out=rows[:], out_offset=None,
                        in_=h_co[:],
                        in_offset=IndirectOffsetOnAxis(
                            ap=idx_tiles[g][:, 0:1], axis=0),
                        bounds_check=T - 1, oob_is_err=False)
                    for k in range(16):
                        tp = ps8.tile([128, 128], BF16, name="tg_ps", tag="t",
                                      space="PSUM")
                        nc.tensor.transpose(tp[:], rows[:, ts(k, 128)],
                                            id128b[:])
                        nc.vector.tensor_copy(xgT[k][:, ts(g, 128)],
                                              tp[:])
                if KDBG:
                    for k in range(16):
                        nc.sync.dma_start(dbg_xg[ts(k, 128), :], xgT[k][:])

            with tc.tile_pool(name="silu_pool", bufs=3) as silu_pool, \
                 tc.tile_pool(name="ps_f", bufs=8, space="PSUM") as ps_f:
                for mb in range(8):
                    w1ts, w3ts = w13_pre.pop(mb)
                    if mb + 2 < 8:
                        w13_pre[mb + 2] = load_w13(mb + 2)
                    for mi in range(4):
                        m = 4 * mb + mi
                        h1_ps = [ps_f.tile([128, NW], F32, name="h1_ps",
                                           tag="t", space="PSUM")
                                 for _ in range(NSPL)]
                        h3_ps = [ps_f.tile([128, NW], F32, name="h3_ps",
                                           tag="t", space="PSUM")
                                 for _ in range(NSPL)]
                        for k in range(16):
                            for s in range(NSPL):
                                nc.tensor.matmul(h1_ps[s][:],
                                                 w1ts[k][:, ts(mi, 128)],
                                                 xgT[k][:, ts(s, NW)],
                                                 start=(k == 0),
                                                 stop=(k == 15))
                            for s in range(NSPL):
                                nc.tensor.matmul(h3_ps[s][:],
                                                 w3ts[k][:, ts(mi, 128)],
                                                 xgT[k][:, ts(s, NW)],
                                                 start=(k == 0),
                                                 stop=(k == 15))
                        gt = g_pool.tile([128, CAP], BF16, name=f"g_{m}")
                        for s in range(NSPL):
                            s1 = silu_pool.tile([128, NW], BF16,
                                                name="silu_t", tag="s")
                            nc.scalar.activation(
                                s1[:], h1_ps[s][:],
                                mybir.ActivationFunctionType.Silu)
                            nc.vector.tensor_mul(gt[:, ts(s, NW)], s1[:],
                                                 h3_ps[s][:])
                        g_tiles.append(gt)
                        if KDBG:
                            nc.sync.dma_start(dbg_g[ts(m, 128), :], gt[:])
        fctx.close()

        # w2 + transpose back + per-chunk scale/scatter + 4-chunk RS
        with tc.tile_pool(name="orow_pool", bufs=1) as orow_pool, \
             tc.tile_pool(name="oe_pool", bufs=2) as oe_pool, \
             tc.tile_pool(name="w2_pool", bufs=48) as w2_pool, \
             tc.tile_pool(name="ps_w", bufs=4, space="PSUM") as ps_w, \
             tc.tile_pool(name="ps_wt", bufs=3, space="PSUM") as ps_wt:
            orows = [orow_pool.tile([128, HID], BF16, name=f"orow_{g}")
                     for g in range(NG)]

            for db in range(4):
                w2ts = []
                for m in range(32):
                    q = nc.sync if (m % 2 == 0) else nc.scalar
                    w2t = w2_pool.tile([128, 512], BF16, name="w2_t", tag="w")
                    q.dma_start(w2t[:], w2_in[ts(m, 128), ts(db, 512)])
                    w2ts.append(w2t)
                for di in range(4):
                    d = 4 * db + di
                    o_ps = [ps_w.tile([128, NW], F32, name="oe_ps", tag="t",
                                      space="PSUM") for _ in range(NSPL)]
                    for m in range(32):
                        for s in range(NSPL):
                            nc.tensor.matmul(o_ps[s][:],
                                             w2ts[m][:, ts(di, 128)],
                                             g_tiles[m][:, ts(s, NW)],
                                             start=(m == 0), stop=(m == 31))
                    oe = oe_pool.tile([128, CAP], BF16, name="oe", tag="oe")
                    for s in range(NSPL):
                        nc.vector.tensor_copy(oe[:, ts(s, NW)], o_ps[s][:])
                    for g in range(NG):
                        tp = ps_wt.tile([128, 128], BF16, name="to_ps",
                                        tag="t", space="PSUM")
                        nc.tensor.transpose(tp[:], oe[:, ts(g, 128)],
                                            id128b[:])
                        nc.vector.tensor_copy(orows[g][:, ts(d, 128)], tp[:])
                # this 512-col chunk of all orows is complete: scale,
                # scatter into its own buffer, and reduce-scatter it
                c0 = db * 512
                for g in range(NG):
                    nc.vector.tensor_scalar_mul(
                        orows[g][:, c0:c0 + 512],
                        orows[g][:, c0:c0 + 512], wg_tiles[g][:])
                    nc.gpsimd.indirect_dma_start(
                        out=moe_q[db][:],
                        out_offset=IndirectOffsetOnAxis(
                            ap=idx_tiles[g][:, 0:1], axis=0),
                        in_=orows[g][:, c0:c0 + 512],
                        in_offset=None,
                        bounds_check=T - 1, oob_is_err=False)
                nc.gpsimd.collective_compute(
                    "ReduceScatter", mybir.AluOpType.add, replica_groups=RG,
                    ins=[moe_q[db][:]], outs=[rs_q[db][:]])
            if KDBG:
                for g in range(NG):
                    nc.sync.dma_start(dbg_or[ts(g, 128), :], orows[g][:])

        # ================= final residual add =================
        with tc.tile_pool(name="fin_pool", bufs=4) as fin_pool:
            for db in range(4):
                for j in range(2):
                    rt = fin_pool.tile([128, 512], BF16, name="rs_t",
                                       tag="r")
                    nc.sync.dma_start(rt[:], rs_q[db][ts(j, 128), :])
                    ft = fin_pool.tile([128, 512], F32, name="fin_t",
                                       tag="f")
                    nc.vector.tensor_add(
                        ft[:], rt[:],
                        resid2[j][:, db * 512:(db + 1) * 512])
                    nc.sync.dma_start(
                        y_out[ts(j, 128), db * 512:(db + 1) * 512], ft[:])

    nc.finalize()
    return nc


def _host_inputs(hidden, positions, norm1_w, norm2_w, wqkv, wo, gate_w, w1, w2,
                 w3):
    f = np.float32
    bf = ml_dtypes.bfloat16
    hidden = np.asarray(hidden, f)
    positions = np.asarray(positions, np.int32)
    norm1_w = np.asarray(norm1_w, f)
    norm2_w = np.asarray(norm2_w, f)
    wqkv = np.asarray(wqkv, f)
    wo = np.asarray(wo, f)
    gate_w = np.asarray(gate_w, f)
    w1 = np.asarray(w1, f)
    w2 = np.asarray(w2, f)
    w3 = np.asarray(w3, f)

    wqkvT = (wqkv * norm1_w[None, :]).T.copy()
    wqkT = np.ascontiguousarray(wqkvT[:, : QS + KVS]).astype(bf)
    wvT = np.ascontiguousarray(wqkvT[:, QS + KVS:]).astype(bf)
    woT = np.ascontiguousarray(wo.T).astype(bf)
    gateT = np.ascontiguousarray((gate_w * norm2_w[None, :]).T)

    half = HD // 2
    inv_freq = 1.0 / (ROPE_THETA ** (np.arange(0, half, dtype=f) * 2.0 / HD))
    ang = positions.astype(f)[:, None] * inv_freq[None, :]
    c = np.cos(ang).T.astype(f)  # [half, T]
    s = np.sin(ang).T.astype(f)
    cosT = np.concatenate([c, c], axis=0).astype(bf)  # [HD, T]
    sinT = np.concatenate([-s, s], axis=0).astype(bf)  # rotate-half sign

    triu128 = np.triu(np.ones((128, 128), f))
    su16 = np.triu(np.ones((16, 16), f), k=1)
    id16 = np.eye(16, dtype=f)
    id128 = np.eye(128, dtype=f)
    id128b = np.eye(128, dtype=f).astype(bf)
    md0 = np.concatenate([triu128, np.ones((128, 128), f)], axis=1).astype(bf)
    md1 = np.concatenate([np.zeros((128, 128), f), triu128],
                         axis=1).astype(bf)
    iota_c = (np.arange(16)[None, :] * 128
              + np.arange(128)[:, None]).astype(np.int32)

    in_maps = []
    for c_ in range(NC):
        sl = slice(c_ * TS, (c_ + 1) * TS)
        bias_c = np.zeros((128, 16), f)
        bias_c[:, 2 * c_:] = NEG  # diagonal + future blocks excluded in pass A
        e_sel = np.zeros((1, NE), f)
        e_sel[0, c_] = 1.0
        in_maps.append({
            "x": np.ascontiguousarray(hidden[sl]),
            "cos_t": np.ascontiguousarray(cosT[:, sl]),
            "sin_t": np.ascontiguousarray(sinT[:, sl]),
            "wqkT": wqkT,
            "wvT": wvT,
            "woT": woT,
            "gateT": gateT,
            "w1T": np.ascontiguousarray(
                (w1[c_] * norm2_w[None, :]).T.astype(bf)),
            "w3T": np.ascontiguousarray(
                (w3[c_] * norm2_w[None, :]).T.astype(bf)),
            "w2T": np.ascontiguousarray(w2[c_].T.astype(bf)),
            "triu128": triu128,
            "su16": su16,
            "id16": id16,
            "id128": id128,
            "id128b": id128b,
            "md0": md0,
            "md1": md1,
            "bias_c": bias_c,
            "e_sel": e_sel,
            "iota_c": iota_c,
        })
    return in_maps


def kernel(hidden_states, positions, norm1_w, norm2_w, wqkv, wo, gate_w, w1,
           w2, w3, _trace=False):
    if "nc" not in _cache:
        _cache["nc"] = build()
    nc = _cache["nc"]
    in_maps = _host_inputs(
        hidden_states, positions, norm1_w, norm2_w, wqkv, wo, gate_w, w1, w2,
        w3)
    res = run_bass_kernel_spmd(nc, in_maps, core_ids=list(range(NC)),
                               trace=_trace)
    _cache["last_result"] = res
    out = np.concatenate([res.results[c]["y"] for c in range(NC)], axis=0)
    return out.astype(np.float32)


# revision 48
# speedup vs baseline: 1.0711x; 1.0711x over previous
"""Mixtral decoder layer (attention + top-2 MoE) on 8 TRN2 NeuronCores.

Self-contained: hardcodes all shapes/sharding. Strategy:
  - token-parallel attention (core c owns tokens [256c, 256c+256))
  - bf16 for all heavy matmuls/collectives, f32 residual + routing path
  - KV AllGather (2D-shaped, bf16) overlapped with Q projection + RoPE +
    diagonal-attention pass (local KV, partials to SBUF)
  - expert-parallel MoE (core c owns expert c), token compaction via
    matmul prefix-sums + indirect DMA scatter/gather, capacity 640
  - w AllGather before h AllGather; selection overlaps the h AllGather
  - MoE FFN with [128,512] weight DMAs, 2-deep mb prefetch
  - column-chunked ReduceScatter overlapped with second half of w2
"""

import os
from contextlib import ExitStack

import numpy as np
import ml_dtypes

KDBG = os.environ.get("KDBG", "0") == "1"

import concourse.mybir as mybir
import concourse.tile as tile
from concourse import bacc
from concourse.bass import IndirectOffsetOnAxis, ts
from concourse.bass_utils import run_bass_kernel_spmd

# ---- problem constants (hardcoded per contract) ----
T = 2048
HID = 2048
N_HEADS = 16
N_KV = 4
HD = 128  # head dim
QS = N_HEADS * HD  # 2048
KVS = N_KV * HD  # 512
FFN = 4096
NE = 8
EPS = 1e-5
ROPE_THETA = 10000.0
NC = 8  # cores
TS = T // NC  # 256 tokens per core
CAP = 640  # expert token capacity (mean 512, observed max ~561)
NEG = -1.0e30
SCALE = HD ** -0.5
H2 = HD // 2
NSPL = 2
NW = CAP // NSPL  # 320
NG = CAP // 128  # 5

F32R = mybir.dt.float32r
F32 = mybir.dt.float32
BF16 = mybir.dt.bfloat16
I32 = mybir.dt.int32

_cache = {}


def build():
    nc = bacc.Bacc("TRN2", num_devices=NC, debug=False)

    # ---------------- I/O ----------------
    x_in = nc.dram_tensor("x", [TS, HID], F32, kind="ExternalInput")
    cos_in = nc.dram_tensor("cos_t", [HD, TS], BF16, kind="ExternalInput")
    sin_in = nc.dram_tensor("sin_t", [HD, TS], BF16, kind="ExternalInput")
    wqk_in = nc.dram_tensor("wqkT", [HID, QS + KVS], BF16, kind="ExternalInput")
    wv_in = nc.dram_tensor("wvT", [HID, KVS], BF16, kind="ExternalInput")
    wo_in = nc.dram_tensor("woT", [QS, HID], BF16, kind="ExternalInput")
    gate_in = nc.dram_tensor("gateT", [HID, NE], F32R, kind="ExternalInput")
    w1_in = nc.dram_tensor("w1T", [HID, FFN], BF16, kind="ExternalInput")
    w3_in = nc.dram_tensor("w3T", [HID, FFN], BF16, kind="ExternalInput")
    w2_in = nc.dram_tensor("w2T", [FFN, HID], BF16, kind="ExternalInput")
    triu_in = nc.dram_tensor("triu128", [128, 128], F32, kind="ExternalInput")
    su16_in = nc.dram_tensor("su16", [16, 16], F32, kind="ExternalInput")
    id16_in = nc.dram_tensor("id16", [16, 16], F32, kind="ExternalInput")
    id128_in = nc.dram_tensor("id128", [128, 128], F32R, kind="ExternalInput")
    id128b_in = nc.dram_tensor("id128b", [128, 128], BF16, kind="ExternalInput")
    md0_in = nc.dram_tensor("md0", [128, 256], BF16, kind="ExternalInput")
    md1_in = nc.dram_tensor("md1", [128, 256], BF16, kind="ExternalInput")
    bias_in = nc.dram_tensor("bias_c", [128, 16], F32, kind="ExternalInput")
    esel_in = nc.dram_tensor("e_sel", [1, NE], F32, kind="ExternalInput")
    iota_in = nc.dram_tensor("iota_c", [128, 16], I32, kind="ExternalInput")

    y_out = nc.dram_tensor("y", [TS, HID], F32, kind="ExternalOutput")
    if KDBG:
        dbg_q = nc.dram_tensor("dbg_q", [16 * 128, 256], BF16,
                               kind="ExternalOutput")
        dbg_k = nc.dram_tensor("dbg_k", [4 * 128, 256], BF16,
                               kind="ExternalOutput")
        dbg_v = nc.dram_tensor("dbg_v", [2 * 128, KVS], BF16,
                               kind="ExternalOutput")
        dbg_att = nc.dram_tensor("dbg_att", [16 * 128, 256], BF16,
                                 kind="ExternalOutput")
        dbg_r2 = nc.dram_tensor("dbg_r2", [TS, HID], F32,
                                kind="ExternalOutput")
        dbg_h2 = nc.dram_tensor("dbg_h2", [TS, HID], F32,
                                kind="ExternalOutput")
        dbg_w = nc.dram_tensor("dbg_w", [TS, NE], F32, kind="ExternalOutput")
        dbg_idx = nc.dram_tensor("dbg_idx", [NG * 128, 1], I32,
                                 kind="ExternalOutput")
        dbg_wg = nc.dram_tensor("dbg_wg", [NG * 128, 1], F32,
                                kind="ExternalOutput")
        dbg_xg = nc.dram_tensor("dbg_xg", [16 * 128, CAP], BF16,
                                kind="ExternalOutput")
        dbg_g = nc.dram_tensor("dbg_g", [32 * 128, CAP], BF16,
                               kind="ExternalOutput")
        dbg_or = nc.dram_tensor("dbg_or", [NG * 128, HID], BF16,
                                kind="ExternalOutput")
        dbg_katt = nc.dram_tensor("dbg_katt", [4 * 128, 256], BF16,
                                  kind="ExternalOutput")
        dbg_vatt = nc.dram_tensor("dbg_vatt", [2 * 128, KVS], BF16,
                                  kind="ExternalOutput")
        dbg_kvco = nc.dram_tensor("dbg_kvco", [128, 2048], BF16,
                                  kind="ExternalOutput")
        dbg_pvb = nc.dram_tensor("dbg_pvb", [16 * 128, 256], F32,
                                 kind="ExternalOutput")
        dbg_rsb = nc.dram_tensor("dbg_rsb", [16, 256], F32,
                                 kind="ExternalOutput")

    # ---------------- internal DRAM (collectives) ----------------
    # kv block per core: rows 0..63 = K (4 kv groups x 16 rows, each [HD,TS]
    # flattened), rows 64..127 = V ([TS, KVS] flattened). 2D shape so the
    # collective parallelizes across partition rows.
    kv_ci = nc.dram_tensor("kv_ci", [128, 2048], BF16)
    kv_co = nc.dram_tensor("kv_co", [NC * 128, 2048], BF16, addr_space="Shared")
    w_ci = nc.dram_tensor("w_ci", [TS, NE], F32)
    w_co = nc.dram_tensor("w_co", [T, NE], F32, addr_space="Shared")
    h_ci = nc.dram_tensor("h_ci", [TS, HID], BF16)
    h_co = nc.dram_tensor("h_co", [T, HID], BF16, addr_space="Shared")
    moe_q = [nc.dram_tensor(f"moe_q{i}", [T, 512], BF16) for i in range(4)]
    rs_q = [nc.dram_tensor(f"rs_q{i}", [TS, 512], BF16) for i in range(4)]
    idx_buf = nc.dram_tensor("idx_buf", [T, 1], I32)
    wcol_d = nc.dram_tensor("wcol_d", [T, 1], F32)

    RG = [list(range(NC))]

    with tile.TileContext(nc, pool_alloc_mode="queue") as tc, \
         ExitStack() as gctx:
        const = gctx.enter_context(tc.tile_pool(name="const", bufs=1))
        np_pool = gctx.enter_context(tc.tile_pool(name="np_pool", bufs=1))
        r2_pool = gctx.enter_context(tc.tile_pool(name="r2_pool", bufs=1))
        xpool = gctx.enter_context(tc.tile_pool(name="xpool", bufs=1))

        # x first on the sync queue so norm1 can start ASAP
        x_tiles = []
        for j in range(2):
            xt = xpool.tile([128, HID], F32, name=f"x_{j}")
            nc.sync.dma_start(xt[:], x_in[ts(j, 128), :])
            x_tiles.append(xt)

        _cq = [0]

        def cdma(name, shape, dt, src):
            t = const.tile(shape, dt, name=name)
            q = nc.sync if _cq[0] % 2 == 0 else nc.scalar
            _cq[0] += 1
            q.dma_start(t[:], src[:])
            return t

        su16 = cdma("su16s", [16, 16], F32, su16_in)
        id16 = cdma("id16s", [16, 16], F32, id16_in)
        id128 = cdma("id128s", [128, 128], F32R, id128_in)
        id128b = cdma("id128bs", [128, 128], BF16, id128b_in)
        md0 = cdma("md0s", [128, 256], BF16, md0_in)
        md1 = cdma("md1s", [128, 256], BF16, md1_in)
        bias_c = cdma("bias_cs", [128, 16], F32, bias_in)
        cosb = cdma("cosbs", [HD, TS], BF16, cos_in)
        sinb = cdma("sinbs", [HD, TS], BF16, sin_in)
        iota_sb = cdma("iota_sbs", [128, 16], I32, iota_in)
        esel = cdma("esels", [1, NE], F32, esel_in)
        triu_f = cdma("triu_fs", [128, 128], F32, triu_in)
        epsb = const.tile([128, 1], F32, name="epsb")
        nc.vector.memset(epsb[:], EPS)
        ones1_f = const.tile([1, 128], F32, name="ones1_f")
        nc.vector.memset(ones1_f[:], 1.0)
        onesp_f = const.tile([128, 1], F32, name="onesp_f")
        nc.vector.memset(onesp_f[:], 1.0)
        onesp_b = const.tile([128, 1], BF16, name="onesp_b")
        nc.vector.tensor_copy(onesp_b[:], onesp_f[:])

        def rms_norm(src_tiles, dst_pool, dst_name, dst_dt):
            out = []
            for j, xt in enumerate(src_tiles):
                scratch = np_pool.tile([128, HID], F32, name="nscratch",
                                       tag="nscratch")
                ssq = np_pool.tile([128, 1], F32, name="nssq", tag="nssq")
                nc.scalar.activation(
                    scratch[:], xt[:], mybir.ActivationFunctionType.Square,
                    accum_out=ssq[:])
                std = np_pool.tile([128, 1], F32, name="nstd", tag="nstd")
                nc.scalar.activation(
                    std[:], ssq[:], mybir.ActivationFunctionType.Sqrt,
                    bias=epsb[:], scale=1.0 / HID)
                rstd = np_pool.tile([128, 1], F32, name="nrstd", tag="nrstd")
                nc.vector.reciprocal(rstd[:], std[:])
                hn = dst_pool.tile([128, HID], dst_dt, name=f"{dst_name}_{j}")
                nc.vector.tensor_scalar_mul(hn[:], xt[:], rstd[:])
                out.append(hn)
            return out

        # ---- idx sentinel early (moe zeroing deferred to attention) ----
        with tc.tile_pool(name="zpool", bufs=1) as zpool:
            zidx = zpool.tile([128, 16], I32, name="zidx")
            nc.vector.memset(zidx[:], 4095)
            nc.gpsimd.dma_start(
                idx_buf[:].rearrange("(j p) one -> p (j one)", p=128),
                zidx[:])

        # ================= phase 1: norm1, X^T =================
        actx = ExitStack()  # pools that live through attention/o_proj
        qkT_pool = actx.enter_context(tc.tile_pool(name="qkT_pool", bufs=1))
        v_pool = actx.enter_context(tc.tile_pool(name="v_pool", bufs=1))
        att_pool = actx.enter_context(tc.tile_pool(name="att_pool", bufs=1))
        pvb_pool = actx.enter_context(tc.tile_pool(name="pvb_pool", bufs=1))

        kT = [None] * N_KV
        qT = [None] * N_HEADS
        v_tiles = []

        with tc.tile_pool(name="hn_pool", bufs=1) as hn_pool, \
             tc.tile_pool(name="xt_pool", bufs=1) as xt_pool, \
             tc.tile_pool(name="wv_pool", bufs=1) as wv_pool, \
             tc.tile_pool(name="wqk_pool", bufs=8) as wqk_pool, \
             tc.tile_pool(name="rope_pool", bufs=4) as rope_pool, \
             tc.tile_pool(name="ps_tp", bufs=2, space="PSUM") as ps_tp, \
             tc.tile_pool(name="ps_mm", bufs=4, space="PSUM") as ps_mm, \
             tc.tile_pool(name="psv", bufs=2, space="PSUM") as psv:
            # prefetch V weights on the gpsimd queue (used after K block)
            wv_tiles = []
            for k in range(16):
                wvt = wv_pool.tile([128, KVS], BF16, name=f"wv_{k}")
                nc.gpsimd.dma_start(wvt[:], wv_in[ts(k, 128), :])
                wv_tiles.append(wvt)

            hn_tiles = rms_norm(x_tiles, hn_pool, "hn", BF16)

            xT = []
            for k in range(16):
                xtile = xt_pool.tile([128, 256], BF16, name=f"xT_{k}")
                for j in range(2):
                    tp = ps_tp.tile([128, 128], BF16, name="tp_ps", tag="tp",
                                    space="PSUM")
                    nc.tensor.transpose(tp[:], hn_tiles[j][:, ts(k, 128)],
                                        id128b[:])
                    nc.vector.tensor_copy(xtile[:, ts(j, 128)], tp[:])
                xT.append(xtile)

            def rope(src):
                rot = rope_pool.tile([128, 256], BF16, name="rrot", tag="rot")
                nc.sync.dma_start(rot[0:H2, :], src[H2:HD, :])
                nc.sync.dma_start(rot[H2:HD, :], src[0:H2, :])
                ta = rope_pool.tile([128, 256], BF16, name="rta", tag="ra")
                tb = rope_pool.tile([128, 256], BF16, name="rtb", tag="rb")
                nc.vector.tensor_mul(ta[:], src[:], cosb[:])
                nc.vector.tensor_mul(tb[:], rot[:], sinb[:])
                return ta, tb

            def proj_block(ob, names):
                # one 512-col output block of wqkT -> 4 [128,256] bf16 tiles
                pss = [ps_mm.tile([128, 256], F32, name="qk_ps", tag="mm",
                                  space="PSUM") for _ in range(4)]
                for k in range(16):
                    q = nc.sync if (k % 2 == 0) else nc.scalar
                    wt = wqk_pool.tile([128, 512], BF16, name="wqk_t", tag="w")
                    q.dma_start(wt[:], wqk_in[ts(k, 128), ts(ob, 512)])
                    for oi in range(4):
                        nc.tensor.matmul(pss[oi][:], wt[:, ts(oi, 128)],
                                         xT[k][:], start=(k == 0),
                                         stop=(k == 15))
                outs = []
                for oi in range(4):
                    dst = qkT_pool.tile([128, 256], BF16, name=names[oi])
                    nc.vector.tensor_copy(dst[:], pss[oi][:])
                    ta, tb = rope(dst)
                    nc.vector.tensor_add(dst[:], ta[:], tb[:])
                    outs.append(dst)
                return outs

            # --- K first (output cols 2048..2560) ---
            kT[0:4] = proj_block(4, [f"kT_{i}" for i in range(4)])

            # --- V ---
            vps = [psv.tile([128, KVS], F32, name="v_ps", tag="v",
                            space="PSUM") for _ in range(2)]
            for k in range(16):
                for j in range(2):
                    nc.tensor.matmul(vps[j][:], xT[k][:, ts(j, 128)],
                                     wv_tiles[k][:], start=(k == 0),
                                     stop=(k == 15))
            for j in range(2):
                vt = v_pool.tile([128, KVS], BF16, name=f"v_{j}")
                nc.vector.tensor_copy(vt[:], vps[j][:])
                v_tiles.append(vt)

            # --- stage K/V and kick the KV AllGather ---
            for kv in range(N_KV):
                nc.sync.dma_start(
                    kv_ci[kv * 16:(kv + 1) * 16, :].rearrange(
                        "a (b t) -> (a b) t", t=TS),
                    kT[kv][:])
            for j in range(2):
                nc.sync.dma_start(
                    kv_ci[64 + 32 * j:64 + 32 * (j + 1), :].rearrange(
                        "a (b d) -> (a b) d", d=KVS),
                    v_tiles[j][:])
            nc.gpsimd.collective_compute(
                "AllGather", mybir.AluOpType.bypass, replica_groups=RG,
                ins=[kv_ci[:]], outs=[kv_co[:]])

            # --- Q (overlaps the AllGather) ---
            for ob in range(4):
                qT[4 * ob:4 * ob + 4] = proj_block(
                    ob, [f"qT_{4 * ob + i}" for i in range(4)])

            if KDBG:
                for o in range(16):
                    nc.sync.dma_start(dbg_q[ts(o, 128), :], qT[o][:])
                for kv in range(N_KV):
                    nc.sync.dma_start(dbg_k[ts(kv, 128), :], kT[kv][:])
                for j in range(2):
                    nc.sync.dma_start(dbg_v[ts(j, 128), :], v_tiles[j][:])

        # ============ pass B: diagonal attention with local KV ============
        pvB = [None] * N_HEADS
        rsB = [None] * N_HEADS
        with tc.tile_pool(name="eb_pool", bufs=4) as eb_pool, \
             tc.tile_pool(name="ps_bs", bufs=2, space="PSUM") as ps_bs, \
             tc.tile_pool(name="ps_bpv", bufs=2, space="PSUM") as ps_bpv, \
             tc.tile_pool(name="ps_brs", bufs=2, space="PSUM") as ps_brs:
            for kv in range(N_KV):
                for hp in range(2):
                    heads = [4 * kv + 2 * hp, 4 * kv + 2 * hp + 1]
                    pv_ps = ps_bpv.tile([128, 512], F32, name="bpv_ps",
                                        tag="pv", space="PSUM")
                    rs_ps = ps_brs.tile([1, 512], F32, name="brs_ps",
                                        tag="rs", space="PSUM")
                    for half in range(2):
                        sps = ps_bs.tile([128, 512], F32, name="bs_ps",
                                         tag="s", space="PSUM")
                        for i in range(2):
                            nc.tensor.matmul(sps[:, ts(i, 256)],
                                             kT[kv][:, ts(half, 128)],
                                             qT[heads[i]][:],
                                             start=True, stop=True)
                        et = eb_pool.tile([128, 512], BF16, name="bet",
                                          tag="e")
                        nc.scalar.activation(
                            et[:], sps[:],
                            mybir.ActivationFunctionType.Exp, scale=SCALE)
                        msk = md0 if half == 0 else md1
                        for i in range(2):
                            nc.vector.tensor_mul(et[:, ts(i, 256)],
                                                 et[:, ts(i, 256)], msk[:])
                        nc.tensor.matmul(pv_ps[:],
                                         v_tiles[half][:, ts(kv, 128)],
                                         et[:], start=(half == 0),
                                         stop=(half == 1))
                        nc.tensor.matmul(rs_ps[:], onesp_b[:], et[:],
                                         start=(half == 0),
                                         stop=(half == 1))
                    for i in range(2):
                        pb = pvb_pool.tile([128, 256], F32,
                                           name=f"pvB_{heads[i]}")
                        nc.vector.tensor_copy(pb[:], pv_ps[:, ts(i, 256)])
                        pvB[heads[i]] = pb
                        rb = pvb_pool.tile([1, 256], F32,
                                           name=f"rsB_{heads[i]}")
                        nc.vector.tensor_copy(rb[:], rs_ps[:, ts(i, 256)])
                        rsB[heads[i]] = rb
                        if KDBG:
                            nc.sync.dma_start(dbg_pvb[ts(heads[i], 128), :],
                                              pb[:])
                            nc.sync.dma_start(
                                dbg_rsb[heads[i]:heads[i] + 1, :], rb[:])

        # ================= pass A: gathered attention =================
        attnT = [None] * N_HEADS
        with tc.tile_pool(name="kvt_pool", bufs=1) as kvt_pool, \
             tc.tile_pool(name="e_pool", bufs=6) as e_pool, \
             tc.tile_pool(name="sc_pool", bufs=4) as sc_pool, \
             tc.tile_pool(name="ps_s", bufs=3, space="PSUM") as ps_s, \
             tc.tile_pool(name="ps_pv", bufs=2, space="PSUM") as ps_pv, \
             tc.tile_pool(name="ps_rs", bufs=2, space="PSUM") as ps_rs:
            katt = [[None] * N_KV for _ in range(NC)]
            vatt = [None] * 16
            qs = [nc.sync, nc.scalar, nc.gpsimd]
            qi = 0
            for r in range(NC):
                for kv in range(N_KV):
                    kt = kvt_pool.tile([128, 256], BF16, name=f"k_{r}_{kv}")
                    qs[qi % 3].dma_start(
                        kt[:],
                        kv_co[128 * r + 16 * kv:128 * r + 16 * (kv + 1), :]
                        .rearrange("a (b t) -> (a b) t", t=TS))
                    katt[r][kv] = kt
                    qi += 1
                for j in range(2):
                    vt = kvt_pool.tile([128, KVS], BF16, name=f"v_{r}_{j}")
                    qs[qi % 3].dma_start(
                        vt[:],
                        kv_co[128 * r + 64 + 32 * j:128 * r + 64 + 32 * (j + 1),
                              :].rearrange("a (b d) -> (a b) d", d=KVS))
                    vatt[2 * r + j] = vt
                    qi += 1

            # zero the moe scatter buffers (DMA engines idle during pass A)
            ztf = kvt_pool.tile([128, 512], F32, name="ztf")
            nc.vector.memset(ztf[:], 0.0)
            ztile = kvt_pool.tile([128, 512], BF16, name="ztile")
            nc.vector.tensor_copy(ztile[:], ztf[:])
            for q in range(4):
                for i in range(T // 128):
                    nc.gpsimd.dma_start(moe_q[q][ts(i, 128), :], ztile[:])

            if KDBG:
                for kv in range(N_KV):
                    nc.sync.dma_start(dbg_katt[ts(kv, 128), :],
                                      katt[0][kv][:])
                for j in range(2):
                    nc.sync.dma_start(dbg_vatt[ts(j, 128), :], vatt[j][:])
                kvraw = kvt_pool.tile([128, 2048], BF16, name="kvraw")
                nc.sync.dma_start(kvraw[:], kv_co[0:128, :])
                nc.sync.dma_start(dbg_kvco[:], kvraw[:])

            for kv in range(N_KV):
                for hp in range(2):
                    heads = [4 * kv + 2 * hp, 4 * kv + 2 * hp + 1]
                    pv_ps = ps_pv.tile([128, 512], F32, name="pv_ps",
                                       tag="pv", space="PSUM")
                    rs_ps = ps_rs.tile([1, 512], F32, name="rs_ps",
                                       tag="rs", space="PSUM")
                    for sg in range(16):
                        sps = ps_s.tile([128, 512], F32, name="s_ps", tag="s",
                                        space="PSUM")
                        for i in range(2):
                            nc.tensor.matmul(sps[:, ts(i, 256)],
                                             katt[sg // 2][kv][:,
                                                              ts(sg % 2, 128)],
                                             qT[heads[i]][:],
                                             start=True, stop=True)
                        et = e_pool.tile([128, 512], BF16, name="et", tag="e")
                        nc.scalar.activation(
                            et[:], sps[:],
                            mybir.ActivationFunctionType.Exp,
                            bias=bias_c[:, sg:sg + 1], scale=SCALE)
                        nc.tensor.matmul(pv_ps[:], vatt[sg][:, ts(kv, 128)],
                                         et[:], start=(sg == 0),
                                         stop=(sg == 15))
                        nc.tensor.matmul(rs_ps[:], onesp_b[:], et[:],
                                         start=(sg == 0), stop=(sg == 15))
                    for i in range(2):
                        h = heads[i]
                        rs_sb = sc_pool.tile([1, 256], F32R, name="rs_sb",
                                             tag="rsb")
                        nc.vector.tensor_add(rs_sb[:], rs_ps[:, ts(i, 256)],
                                             rsB[h][:])
                        with nc.allow_low_precision(
                                reason="f32r recip for PE bcast"):
                            nc.vector.reciprocal(rs_sb[:], rs_sb[:])
                        bc_ps = ps_s.tile([128, 256], F32, name="bc_ps",
                                          tag="s", space="PSUM")
                        nc.tensor.matmul(bc_ps[:], ones1_f[:].bitcast(F32R),
                                         rs_sb[:], start=True, stop=True)
                        bc_sb = sc_pool.tile([128, 256], F32, name="bc_sb",
                                             tag="bcs")
                        nc.scalar.copy(bc_sb[:], bc_ps[:])
                        pvt = sc_pool.tile([128, 256], F32, name="pv_tot",
                                           tag="pvt")
                        nc.vector.tensor_add(pvt[:], pv_ps[:, ts(i, 256)],
                                             pvB[h][:])
                        at = att_pool.tile([128, 256], BF16,
                                           name=f"attnT_{h}")
                        nc.vector.tensor_mul(at[:], pvt[:], bc_sb[:])
                        attnT[h] = at

        # ============ o_proj + residual ============
        resid2 = []
        with tc.tile_pool(name="wo_pool", bufs=5) as wo_pool, \
             tc.tile_pool(name="ps5", bufs=8, space="PSUM") as ps5:
            o_ps = [[ps5.tile([128, 512], F32, name="o_ps", tag="t",
                              space="PSUM") for _ in range(4)]
                    for _ in range(2)]
            for k in range(16):
                q = nc.sync if (k % 2 == 0) else nc.scalar
                wt = wo_pool.tile([128, HID], BF16, name="wo_t", tag="w")
                q.dma_start(wt[:], wo_in[ts(k, 128), :])
                for j in range(2):
                    for nb in range(4):
                        nc.tensor.matmul(o_ps[j][nb][:],
                                         attnT[k][:, ts(j, 128)],
                                         wt[:, ts(nb, 512)], start=(k == 0),
                                         stop=(k == 15))
            for j in range(2):
                r2 = r2_pool.tile([128, HID], F32, name=f"resid2_{j}")
                for nb in range(4):
                    nc.vector.tensor_add(r2[:, ts(nb, 512)], o_ps[j][nb][:],
                                         x_tiles[j][:, ts(nb, 512)])
                resid2.append(r2)
        if KDBG:
            for h in range(16):
                nc.sync.dma_start(dbg_att[ts(h, 128), :], attnT[h][:])
            for j in range(2):
                nc.sync.dma_start(dbg_r2[ts(j, 128), :], resid2[j][:])
        actx.close()

        # ============ norm2 + gate + fused h/w AllGather ============
        with tc.tile_pool(name="h2_pool", bufs=1) as h2_pool:
            h2n_tiles = rms_norm(resid2, h2_pool, "h2n", F32R)

            with tc.tile_pool(name="x2t_pool", bufs=1) as x2t_pool, \
                 tc.tile_pool(name="gate_pool", bufs=2) as gate_pool, \
                 tc.tile_pool(name="ps6t", bufs=2, space="PSUM") as ps6t, \
                 tc.tile_pool(name="ps6b", bufs=2, space="PSUM") as ps6b:
                x2T = []
                for k in range(16):
                    row = []
                    for j in range(2):
                        dst = x2t_pool.tile([128, 128], F32R,
                                            name=f"x2T_{k}_{j}")
                        tp = ps6t.tile([128, 128], F32R, name="tp2_ps",
                                       tag="t", space="PSUM")
                        nc.tensor.transpose(tp[:],
                                            h2n_tiles[j][:, ts(k, 128)],
                                            id128[:])
                        nc.vector.tensor_copy(dst[:], tp[:])
                        row.append(dst)
                    x2T.append(row)

                gsb = gate_pool.tile([128, 16 * NE], F32R, name="gsb")
                nc.sync.dma_start(
                    gsb[:].rearrange("p (k e) -> p k e", e=NE),
                    gate_in[:].rearrange("(k p) e -> p k e", p=128))
                for j in range(2):
                    gps = ps6b.tile([128, NE], F32, name="g_ps", tag="t",
                                    space="PSUM")
                    for k in range(16):
                        nc.tensor.matmul(
                            gps[:], x2T[k][j][:],
                            gsb[:].rearrange("p (k e) -> p k e", e=NE)[:, k, :],
                            start=(k == 0), stop=(k == 15))
                    lg = gate_pool.tile([128, NE], F32, name="lg", tag="g1")
                    nc.vector.tensor_copy(lg[:], gps[:])
                    mx = gate_pool.tile([128, 1], F32, name="gmx", tag="g2")
                    nc.vector.reduce_max(mx[:], lg[:],
                                         axis=mybir.AxisListType.X)
                    nmx = gate_pool.tile([128, 1], F32, name="gnmx", tag="g3")
                    nc.vector.tensor_scalar_mul(nmx[:], mx[:], -1.0)
                    p = gate_pool.tile([128, NE], F32, name="gp", tag="g4")
                    nc.scalar.activation(p[:], lg[:],
                                         mybir.ActivationFunctionType.Exp,
                                         bias=nmx[:])
                    v1 = gate_pool.tile([128, 1], F32, name="gv1", tag="g5")
                    nc.vector.reduce_max(v1[:], p[:],
                                         axis=mybir.AxisListType.X)
                    ge1 = gate_pool.tile([128, NE], F32, name="gge1", tag="g6")
                    nc.vector.tensor_single_scalar(ge1[:], p[:], v1[:],
                                                   op=mybir.AluOpType.is_ge)
                    pt = gate_pool.tile([128, NE], F32, name="gpt", tag="g7")
                    nc.vector.tensor_mul(pt[:], p[:], ge1[:])
                    p2 = gate_pool.tile([128, NE], F32, name="gp2", tag="g8")
                    nc.vector.tensor_sub(p2[:], p[:], pt[:])
                    v2 = gate_pool.tile([128, 1], F32, name="gv2", tag="g9")
                    nc.vector.reduce_max(v2[:], p2[:],
                                         axis=mybir.AxisListType.X)
                    m2 = gate_pool.tile([128, NE], F32, name="gm2", tag="g10")
                    nc.vector.tensor_single_scalar(m2[:], p[:], v2[:],
                                                   op=mybir.AluOpType.is_ge)
                    pm = gate_pool.tile([128, NE], F32, name="gpm", tag="g11")
                    nc.vector.tensor_mul(pm[:], p[:], m2[:])
                    s12 = gate_pool.tile([128, 1], F32, name="gs12", tag="g12")
                    nc.vector.tensor_add(s12[:], v1[:], v2[:])
                    nc.vector.reciprocal(s12[:], s12[:])
                    wful = h2_pool.tile([128, NE], F32, name=f"wful_{j}")
                    nc.vector.tensor_scalar_mul(wful[:], pm[:], s12[:])
                    nc.sync.dma_start(w_ci[ts(j, 128), :], wful[:])
                    if KDBG:
                        nc.sync.dma_start(dbg_w[ts(j, 128), :], wful[:])

                nc.gpsimd.collective_compute(
                    "AllGather", mybir.AluOpType.bypass, replica_groups=RG,
                    ins=[w_ci[:]], outs=[w_co[:]])

            # h cast + stage + AllGather (selection overlaps this)
            with tc.tile_pool(name="h2b_pool", bufs=2) as h2b_pool:
                for j in range(2):
                    hb = h2b_pool.tile([128, HID], BF16, name="h2b", tag="b")
                    nc.vector.tensor_copy(hb[:], h2n_tiles[j][:])
                    nc.sync.dma_start(h_ci[ts(j, 128), :], hb[:])
                    if KDBG:
                        nc.sync.dma_start(dbg_h2[ts(j, 128), :],
                                          h2n_tiles[j][:].bitcast(F32))
            nc.gpsimd.collective_compute(
                "AllGather", mybir.AluOpType.bypass, replica_groups=RG,
                ins=[h_ci[:]], outs=[h_co[:]])

        # ====== FFN weight prefetch (overlaps AllGather + selection) ======
        gat_pool = gctx.enter_context(tc.tile_pool(name="gat_pool", bufs=1))
        g_pool = gctx.enter_context(tc.tile_pool(name="g_pool", bufs=1))
        fctx = ExitStack()
        w13_pool = fctx.enter_context(tc.tile_pool(name="w13_pool", bufs=40))

        def load_w13(mb):
            w1ts, w3ts = [], []
            for k in range(16):
                w1t = w13_pool.tile([128, 512], BF16, name="w1_t", tag="w1")
                nc.sync.dma_start(w1t[:], w1_in[ts(k, 128), ts(mb, 512)])
                w1ts.append(w1t)
                w3t = w13_pool.tile([128, 512], BF16, name="w3_t", tag="w3")
                nc.scalar.dma_start(w3t[:], w3_in[ts(k, 128), ts(mb, 512)])
                w3ts.append(w3t)
            return w1ts, w3ts

        w13_pre = {mb: load_w13(mb) for mb in range(2)}

        # ================= expert token selection =================
        with tc.tile_pool(name="sel_pool", bufs=1) as sel_pool, \
             tc.tile_pool(name="ps7", bufs=2, space="PSUM") as ps7:
            wall = sel_pool.tile([128, 16 * NE], F32, name="wall")
            nc.sync.dma_start(
                wall[:].rearrange("p (k e) -> p k e", e=NE),
                w_co[:].rearrange("(k p) e -> p k e", p=128))
            eselb = sel_pool.tile([128, NE], F32, name="eselb")
            nc.gpsimd.partition_broadcast(eselb[:], esel[:])
            wsel = sel_pool.tile([128, 16 * NE], F32, name="wsel")
            nc.vector.tensor_tensor(
                wsel[:].rearrange("p (k e) -> p k e", e=NE),
                wall[:].rearrange("p (k e) -> p k e", e=NE),
                eselb[:].rearrange("p (o e) -> p o e", o=1)
                .to_broadcast([128, 16, NE]),
                op=mybir.AluOpType.mult)
            wcol = sel_pool.tile([128, 16], F32, name="wcol")
            nc.vector.reduce_sum(
                wcol[:], wsel[:].rearrange("p (k e) -> p k e", e=NE),
                axis=mybir.AxisListType.X)
            nc.sync.dma_start(
                wcol_d[:].rearrange("(j p) one -> p (j one)", p=128), wcol[:])
            mall = sel_pool.tile([128, 16], F32, name="mall")
            nc.vector.tensor_single_scalar(mall[:], wcol[:], 0.0,
                                           op=mybir.AluOpType.is_gt)
            rank_ps = ps7.tile([128, 16], F32, name="rank_ps", tag="a",
                               space="PSUM")
            nc.tensor.matmul(rank_ps[:], triu_f[:], mall[:], start=True,
                             stop=True)
            tot_ps = ps7.tile([1, 16], F32, name="tot_ps", tag="b",
                              space="PSUM")
            nc.tensor.matmul(tot_ps[:], onesp_f[:], mall[:], start=True,
                             stop=True)
            tot = sel_pool.tile([1, 16], F32, name="tot")
            nc.vector.tensor_copy(tot[:], tot_ps[:])
            totT_ps = ps7.tile([16, 1], F32, name="totT_ps", tag="b",
                               space="PSUM")
            nc.tensor.matmul(totT_ps[:], tot[:], ones1_f[:, 0:1], start=True,
                             stop=True)
            totT = sel_pool.tile([16, 1], F32, name="totT")
            nc.vector.tensor_copy(totT[:], totT_ps[:])
            ex_ps = ps7.tile([16, 1], F32, name="ex_ps", tag="b", space="PSUM")
            nc.tensor.matmul(ex_ps[:], su16[:], totT[:], start=True, stop=True)
            exT = sel_pool.tile([16, 1], F32, name="exT")
            nc.vector.tensor_copy(exT[:], ex_ps[:])
            exr_ps = ps7.tile([1, 16], F32, name="exr_ps", tag="b",
                              space="PSUM")
            nc.tensor.matmul(exr_ps[:], exT[:], id16[:], start=True, stop=True)
            exr = sel_pool.tile([1, 16], F32, name="exr")
            nc.vector.tensor_copy(exr[:], exr_ps[:])
            exb_ps = ps7.tile([128, 16], F32, name="exb_ps", tag="b",
                              space="PSUM")
            nc.tensor.matmul(exb_ps[:], ones1_f[:], exr[:], start=True,
                             stop=True)
            posf = sel_pool.tile([128, 16], F32, name="posf")
            nc.vector.tensor_copy(posf[:], rank_ps[:])
            nc.vector.tensor_add(posf[:], posf[:], exb_ps[:])
            adj = sel_pool.tile([128, 16], F32, name="adj")
            nc.vector.tensor_scalar(
                adj[:], mall[:], -4096.0, 4095.0,
                op0=mybir.AluOpType.mult, op1=mybir.AluOpType.add)
            nc.vector.tensor_add(posf[:], posf[:], adj[:])
            posi = sel_pool.tile([128, 16], I32, name="posi")
            nc.vector.tensor_copy(posi[:], posf[:])
            for j in range(16):
                nc.gpsimd.indirect_dma_start(
                    out=idx_buf[:],
                    out_offset=IndirectOffsetOnAxis(ap=posi[:, j:j + 1],
                                                    axis=0),
                    in_=iota_sb[:, j:j + 1],
                    in_offset=None,
                    bounds_check=T - 1, oob_is_err=False)

        idx_tiles, wg_tiles = [], []
        for g in range(NG):
            it = gat_pool.tile([128, 1], I32, name=f"idx_{g}")
            nc.sync.dma_start(it[:], idx_buf[ts(g, 128), :])
            idx_tiles.append(it)
            wg = gat_pool.tile([128, 1], F32, name=f"wg_{g}")
            nc.vector.memset(wg[:], 0.0)
            nc.gpsimd.indirect_dma_start(
                out=wg[:], out_offset=None,
                in_=wcol_d[:],
                in_offset=IndirectOffsetOnAxis(ap=it[:, 0:1], axis=0),
                bounds_check=T - 1, oob_is_err=False)
            wg_tiles.append(wg)
            if KDBG:
                nc.sync.dma_start(dbg_idx[ts(g, 128), :], it[:])
                nc.sync.dma_start(dbg_wg[ts(g, 128), :], wg[:])

        # ================= gather + expert FFN =================
        g_tiles = []

        with tc.tile_pool(name="xgt_pool", bufs=1) as xgt_pool:
            xgT = [xgt_pool.tile([128, CAP], BF16, name=f"xgT_{k}")
                   for k in range(16)]
            with tc.tile_pool(name="row_pool", bufs=2) as row_pool, \
                 tc.tile_pool(name="ps8", bufs=3, space="PSUM") as ps8:
                for g in range(NG):
                    rows = row_pool.tile([128, HID], BF16, name="xg_rows",
                                         tag="rows")
                    nc.gpsimd.indirect_dma_start(
                        out=rows[:], out_offset=None,
                        in_=h_co[:],
                        in_offset=IndirectOffsetOnAxis(
                            ap=idx_tiles[g][:, 0:1], axis=0),
                        bounds_check=T - 1, oob_is_err=False)
                    for k in range(16):
                        tp = ps8.tile([128, 128], BF16, name="tg_ps", tag="t",
                                      space="PSUM")
                        nc.tensor.transpose(tp[:], rows[:, ts(k, 128)],
                                            id128b[:])
                        nc.vector.tensor_copy(xgT[k][:, ts(g, 128)],
                                              tp[:])
                if KDBG:
                    for k in range(16):
                        nc.sync.dma_start(dbg_xg[ts(k, 128), :], xgT[k][:])

            with tc.tile_pool(name="silu_pool", bufs=3) as silu_pool, \
                 tc.tile_pool(name="ps_f", bufs=8, space="PSUM") as ps_f:
                for mb in range(8):
                    w1ts, w3ts = w13_pre.pop(mb)
                    if mb + 2 < 8:
                        w13_pre[mb + 2] = load_w13(mb + 2)
                    for mi in range(4):
                        m = 4 * mb + mi
                        h1_ps = [ps_f.tile([128, NW], F32, name="h1_ps",
                                           tag="t", space="PSUM")
                                 for _ in range(NSPL)]
                        h3_ps = [ps_f.tile([128, NW], F32, name="h3_ps",
                                           tag="t", space="PSUM")
                                 for _ in range(NSPL)]
                        for k in range(16):
                            for s in range(NSPL):
                                nc.tensor.matmul(h1_ps[s][:],
                                                 w1ts[k][:, ts(mi, 128)],
                                                 xgT[k][:, ts(s, NW)],
                                                 start=(k == 0),
                                                 stop=(k == 15))
                            for s in range(NSPL):
                                nc.tensor.matmul(h3_ps[s][:],
                                                 w3ts[k][:, ts(mi, 128)],
                                                 xgT[k][:, ts(s, NW)],
                                                 start=(k == 0),
                                                 stop=(k == 15))
                        gt = g_pool.tile([128, CAP], BF16, name=f"g_{m}")
                        for s in range(NSPL):
                            s1 = silu_pool.tile([128, NW], BF16,
                                                name="silu_t", tag="s")
                            nc.scalar.activation(
                                s1[:], h1_ps[s][:],
                                mybir.ActivationFunctionType.Silu)
                            nc.vector.tensor_mul(gt[:, ts(s, NW)], s1[:],
                                                 h3_ps[s][:])
                        g_tiles.append(gt)
                        if KDBG:
                            nc.sync.dma_start(dbg_g[ts(m, 128), :], gt[:])
        fctx.close()

        # w2 + transpose back + per-chunk scale/scatter + 4-chunk RS
        with tc.tile_pool(name="orow_pool", bufs=1) as orow_pool, \
             tc.tile_pool(name="oe_pool", bufs=2) as oe_pool, \
             tc.tile_pool(name="w2_pool", bufs=48) as w2_pool, \
             tc.tile_pool(name="ps_w", bufs=4, space="PSUM") as ps_w, \
             tc.tile_pool(name="ps_wt", bufs=3, space="PSUM") as ps_wt:
            orows = [orow_pool.tile([128, HID], BF16, name=f"orow_{g}")
                     for g in range(NG)]

            for db in range(4):
                w2ts = []
                for m in range(32):
                    q = nc.sync if (m % 2 == 0) else nc.scalar
                    w2t = w2_pool.tile([128, 512], BF16, name="w2_t", tag="w")
                    q.dma_start(w2t[:], w2_in[ts(m, 128), ts(db, 512)])
                    w2ts.append(w2t)
                for di in range(4):
                    d = 4 * db + di
                    o_ps = [ps_w.tile([128, NW], F32, name="oe_ps", tag="t",
                                      space="PSUM") for _ in range(NSPL)]
                    for m in range(32):
                        for s in range(NSPL):
                            nc.tensor.matmul(o_ps[s][:],
                                             w2ts[m][:, ts(di, 128)],
                                             g_tiles[m][:, ts(s, NW)],
                                             start=(m == 0), stop=(m == 31))
                    oe = oe_pool.tile([128, CAP], BF16, name="oe", tag="oe")
                    for s in range(NSPL):
                        nc.vector.tensor_copy(oe[:, ts(s, NW)], o_ps[s][:])
                    for g in range(NG):
                        tp = ps_wt.tile([128, 128], BF16, name="to_ps",
                                        tag="t", space="PSUM")
                        nc.tensor.transpose(tp[:], oe[:, ts(g, 128)],
                                            id128b[:])
                        nc.vector.tensor_copy(orows[g][:, ts(d, 128)], tp[:])
                # this 512-col chunk of all orows is complete: scale,
                # scatter into its own buffer, and reduce-scatter it
                c0 = db * 512
                for g in range(NG):
                    nc.vector.tensor_scalar_mul(
                        orows[g][:, c0:c0 + 512],
                        orows[g][:, c0:c0 + 512], wg_tiles[g][:])
                    nc.gpsimd.indirect_dma_start(
                        out=moe_q[db][:],
                        out_offset=IndirectOffsetOnAxis(
                            ap=idx_tiles[g][:, 0:1], axis=0),
                        in_=orows[g][:, c0:c0 + 512],
                        in_offset=None,
                        bounds_check=T - 1, oob_is_err=False)
                nc.gpsimd.collective_compute(
                    "ReduceScatter", mybir.AluOpType.add, replica_groups=RG,
                    ins=[moe_q[db][:]], outs=[rs_q[db][:]])
            if KDBG:
                for g in range(NG):
                    nc.sync.dma_start(dbg_or[ts(g, 128), :], orows[g][:])

        # ================= final residual add =================
        with tc.tile_pool(name="fin_pool", bufs=4) as fin_pool:
            for db in range(4):
                for j in range(2):
                    rt = fin_pool.tile([128, 512], BF16, name="rs_t",
                                       tag="r")
                    nc.sync.dma_start(rt[:], rs_q[db][ts(j, 128), :])
                    ft = fin_pool.tile([128, 512], F32, name="fin_t",
                                       tag="f")
                    nc.vector.tensor_add(
                        ft[:], rt[:],
                        resid2[j][:, db * 512:(db + 1) * 512])
                    nc.sync.dma_start(
                        y_out[ts(j, 128), db * 512:(db + 1) * 512], ft[:])

    nc.finalize()
    return nc


def _host_inputs(hidden, positions, norm1_w, norm2_w, wqkv, wo, gate_w, w1, w2,
                 w3):
    f = np.float32
    bf = ml_dtypes.bfloat16
    hidden = np.asarray(hidden, f)
    positions = np.asarray(positions, np.int32)
    norm1_w = np.asarray(norm1_w, f)
    norm2_w = np.asarray(norm2_w, f)
    wqkv = np.asarray(wqkv, f)
    wo = np.asarray(wo, f)
    gate_w = np.asarray(gate_w, f)
    w1 = np.asarray(w1, f)
    w2 = np.asarray(w2, f)
    w3 = np.asarray(w3, f)

    wqkvT = (wqkv * norm1_w[None, :]).T.copy()
    wqkT = np.ascontiguousarray(wqkvT[:, : QS + KVS]).astype(bf)
    wvT = np.ascontiguousarray(wqkvT[:, QS + KVS:]).astype(bf)
    woT = np.ascontiguousarray(wo.T).astype(bf)
    gateT = np.ascontiguousarray((gate_w * norm2_w[None, :]).T)

    half = HD // 2
    inv_freq = 1.0 / (ROPE_THETA ** (np.arange(0, half, dtype=f) * 2.0 / HD))
    ang = positions.astype(f)[:, None] * inv_freq[None, :]
    c = np.cos(ang).T.astype(f)  # [half, T]
    s = np.sin(ang).T.astype(f)
    cosT = np.concatenate([c, c], axis=0).astype(bf)  # [HD, T]
    sinT = np.concatenate([-s, s], axis=0).astype(bf)  # rotate-half sign

    triu128 = np.triu(np.ones((128, 128), f))
    su16 = np.triu(np.ones((16, 16), f), k=1)
    id16 = np.eye(16, dtype=f)
    id128 = np.eye(128, dtype=f)
    id128b = np.eye(128, dtype=f).astype(bf)
    md0 = np.concatenate([triu128, np.ones((128, 128), f)], axis=1).astype(bf)
    md1 = np.concatenate([np.zeros((128, 128), f), triu128],
                         axis=1).astype(bf)
    iota_c = (np.arange(16)[None, :] * 128
              + np.arange(128)[:, None]).astype(np.int32)

    in_maps = []
    for c_ in range(NC):
        sl = slice(c_ * TS, (c_ + 1) * TS)
        bias_c = np.zeros((128, 16), f)
        bias_c[:, 2 * c_:] = NEG  # diagonal + future blocks excluded in pass A
        e_sel = np.zeros((1, NE), f)
        e_sel[0, c_] = 1.0
        in_maps.append({
            "x": np.ascontiguousarray(hidden[sl]),
            "cos_t": np.ascontiguousarray(cosT[:, sl]),
            "sin_t": np.ascontiguousarray(sinT[:, sl]),
            "wqkT": wqkT,
            "wvT": wvT,
            "woT": woT,
            "gateT": gateT,
            "w1T": np.ascontiguousarray(
                (w1[c_] * norm2_w[None, :]).T.astype(bf)),
            "w3T": np.ascontiguousarray(
                (w3[c_] * norm2_w[None, :]).T.astype(bf)),
            "w2T": np.ascontiguousarray(w2[c_].T.astype(bf)),
            "triu128": triu128,
            "su16": su16,
            "id16": id16,
            "id128": id128,
            "id128b": id128b,
            "md0": md0,
            "md1": md1,
            "bias_c": bias_c,
            "e_sel": e_sel,
            "iota_c": iota_c,
        })
    return in_maps


def kernel(hidden_states, positions, norm1_w, norm2_w, wqkv, wo, gate_w, w1,
           w2, w3, _trace=False):
    if "nc" not in _cache:
        _cache["nc"] = build()
    nc = _cache["nc"]
    in_maps = _host_inputs(
        hidden_states, positions, norm1_w, norm2_w, wqkv, wo, gate_w, w1, w2,
        w3)
    res = run_bass_kernel_spmd(nc, in_maps, core_ids=list(range(NC)),
                               trace=_trace)
    _cache["last_result"] = res
    out = np.concatenate([res.results[c]["y"] for c in range(NC)], axis=0)
    return out.astype(np.float32)


# revision 50
# speedup vs baseline: 1.0962x; 1.0234x over previous
"""Mixtral decoder layer (attention + top-2 MoE) on 8 TRN2 NeuronCores.

Self-contained: hardcodes all shapes/sharding. Strategy:
  - token-parallel attention (core c owns tokens [256c, 256c+256))
  - bf16 for all heavy matmuls/collectives, f32 residual + routing path
  - KV AllGather (2D-shaped, bf16) overlapped with Q projection + RoPE +
    diagonal-attention pass (local KV, partials to SBUF)
  - expert-parallel MoE (core c owns expert c), token compaction via
    matmul prefix-sums + indirect DMA scatter/gather, capacity 640
  - w AllGather before h AllGather; selection overlaps the h AllGather
  - MoE FFN with [128,512] weight DMAs, 2-deep mb prefetch
  - column-chunked ReduceScatter overlapped with second half of w2
"""

import os
from contextlib import ExitStack

import numpy as np
import ml_dtypes

KDBG = os.environ.get("KDBG", "0") == "1"

import concourse.mybir as mybir
import concourse.tile as tile
from concourse import bacc
from concourse.bass import IndirectOffsetOnAxis, ts
from concourse.bass_utils import run_bass_kernel_spmd

# ---- problem constants (hardcoded per contract) ----
T = 2048
HID = 2048
N_HEADS = 16
N_KV = 4
HD = 128  # head dim
QS = N_HEADS * HD  # 2048
KVS = N_KV * HD  # 512
FFN = 4096
NE = 8
EPS = 1e-5
ROPE_THETA = 10000.0
NC = 8  # cores
TS = T // NC  # 256 tokens per core
CAP = 640  # expert token capacity (mean 512, observed max ~561)
NEG = -1.0e30
SCALE = HD ** -0.5
H2 = HD // 2
NSPL = 2
NW = CAP // NSPL  # 320
NG = CAP // 128  # 5

F32R = mybir.dt.float32r
F32 = mybir.dt.float32
BF16 = mybir.dt.bfloat16
I32 = mybir.dt.int32

_cache = {}


def build():
    nc = bacc.Bacc("TRN2", num_devices=NC, debug=False)

    # ---------------- I/O ----------------
    x_in = nc.dram_tensor("x", [TS, HID], F32, kind="ExternalInput")
    cos_in = nc.dram_tensor("cos_t", [HD, TS], BF16, kind="ExternalInput")
    sin_in = nc.dram_tensor("sin_t", [HD, TS], BF16, kind="ExternalInput")
    wqk_in = nc.dram_tensor("wqkT", [HID, QS + KVS], BF16, kind="ExternalInput")
    wv_in = nc.dram_tensor("wvT", [HID, KVS], BF16, kind="ExternalInput")
    wo_in = nc.dram_tensor("woT", [QS, HID], BF16, kind="ExternalInput")
    gate_in = nc.dram_tensor("gateT", [HID, NE], F32R, kind="ExternalInput")
    w1_in = nc.dram_tensor("w1T", [HID, FFN], BF16, kind="ExternalInput")
    w3_in = nc.dram_tensor("w3T", [HID, FFN], BF16, kind="ExternalInput")
    w2_in = nc.dram_tensor("w2T", [FFN, HID], BF16, kind="ExternalInput")
    triu_in = nc.dram_tensor("triu128", [128, 128], F32, kind="ExternalInput")
    su16_in = nc.dram_tensor("su16", [16, 16], F32, kind="ExternalInput")
    id16_in = nc.dram_tensor("id16", [16, 16], F32, kind="ExternalInput")
    id128_in = nc.dram_tensor("id128", [128, 128], F32R, kind="ExternalInput")
    id128b_in = nc.dram_tensor("id128b", [128, 128], BF16, kind="ExternalInput")
    md0_in = nc.dram_tensor("md0", [128, 256], BF16, kind="ExternalInput")
    md1_in = nc.dram_tensor("md1", [128, 256], BF16, kind="ExternalInput")
    bias_in = nc.dram_tensor("bias_c", [128, 16], F32, kind="ExternalInput")
    esel_in = nc.dram_tensor("e_sel", [1, NE], F32, kind="ExternalInput")
    iota_in = nc.dram_tensor("iota_c", [128, 16], I32, kind="ExternalInput")

    y_out = nc.dram_tensor("y", [TS, HID], F32, kind="ExternalOutput")
    if KDBG:
        dbg_q = nc.dram_tensor("dbg_q", [16 * 128, 256], BF16,
                               kind="ExternalOutput")
        dbg_k = nc.dram_tensor("dbg_k", [4 * 128, 256], BF16,
                               kind="ExternalOutput")
        dbg_v = nc.dram_tensor("dbg_v", [2 * 128, KVS], BF16,
                               kind="ExternalOutput")
        dbg_att = nc.dram_tensor("dbg_att", [16 * 128, 256], BF16,
                                 kind="ExternalOutput")
        dbg_r2 = nc.dram_tensor("dbg_r2", [TS, HID], F32,
                                kind="ExternalOutput")
        dbg_h2 = nc.dram_tensor("dbg_h2", [TS, HID], F32,
                                kind="ExternalOutput")
        dbg_w = nc.dram_tensor("dbg_w", [TS, NE], F32, kind="ExternalOutput")
        dbg_idx = nc.dram_tensor("dbg_idx", [NG * 128, 1], I32,
                                 kind="ExternalOutput")
        dbg_wg = nc.dram_tensor("dbg_wg", [NG * 128, 1], F32,
                                kind="ExternalOutput")
        dbg_xg = nc.dram_tensor("dbg_xg", [16 * 128, CAP], BF16,
                                kind="ExternalOutput")
        dbg_g = nc.dram_tensor("dbg_g", [32 * 128, CAP], BF16,
                               kind="ExternalOutput")
        dbg_or = nc.dram_tensor("dbg_or", [NG * 128, HID], BF16,
                                kind="ExternalOutput")
        dbg_katt = nc.dram_tensor("dbg_katt", [4 * 128, 256], BF16,
                                  kind="ExternalOutput")
        dbg_vatt = nc.dram_tensor("dbg_vatt", [2 * 128, KVS], BF16,
                                  kind="ExternalOutput")
        dbg_kvco = nc.dram_tensor("dbg_kvco", [128, 2048], BF16,
                                  kind="ExternalOutput")
        dbg_pvb = nc.dram_tensor("dbg_pvb", [16 * 128, 256], F32,
                                 kind="ExternalOutput")
        dbg_rsb = nc.dram_tensor("dbg_rsb", [16, 256], F32,
                                 kind="ExternalOutput")

    # ---------------- internal DRAM (collectives) ----------------
    # kv block per core: rows 0..63 = K (4 kv groups x 16 rows, each [HD,TS]
    # flattened), rows 64..127 = V ([TS, KVS] flattened). 2D shape so the
    # collective parallelizes across partition rows.
    kv_ci = nc.dram_tensor("kv_ci", [128, 2048], BF16)
    kv_co = nc.dram_tensor("kv_co", [NC * 128, 2048], BF16, addr_space="Shared")
    w_ci = nc.dram_tensor("w_ci", [TS, NE], F32)
    w_co = nc.dram_tensor("w_co", [T, NE], F32, addr_space="Shared")
    h_ci = nc.dram_tensor("h_ci", [TS, HID], BF16)
    h_co = nc.dram_tensor("h_co", [T, HID], BF16, addr_space="Shared")
    moe_q = [nc.dram_tensor(f"moe_q{i}", [T, 512], BF16) for i in range(4)]
    rs_q = [nc.dram_tensor(f"rs_q{i}", [TS, 512], BF16) for i in range(4)]
    idx_buf = nc.dram_tensor("idx_buf", [T, 1], I32)
    wcol_d = nc.dram_tensor("wcol_d", [T, 1], F32)

    RG = [list(range(NC))]

    with tile.TileContext(nc, pool_alloc_mode="queue") as tc, \
         ExitStack() as gctx:
        const = gctx.enter_context(tc.tile_pool(name="const", bufs=1))
        np_pool = gctx.enter_context(tc.tile_pool(name="np_pool", bufs=1))
        r2_pool = gctx.enter_context(tc.tile_pool(name="r2_pool", bufs=1))
        xpool = gctx.enter_context(tc.tile_pool(name="xpool", bufs=1))

        # x first on the sync queue so norm1 can start ASAP
        x_tiles = []
        for j in range(2):
            xt = xpool.tile([128, HID], F32, name=f"x_{j}")
            nc.sync.dma_start(xt[:], x_in[ts(j, 128), :])
            x_tiles.append(xt)

        _cq = [0]

        def cdma(name, shape, dt, src):
            t = const.tile(shape, dt, name=name)
            q = nc.sync if _cq[0] % 2 == 0 else nc.scalar
            _cq[0] += 1
            q.dma_start(t[:], src[:])
            return t

        su16 = cdma("su16s", [16, 16], F32, su16_in)
        id16 = cdma("id16s", [16, 16], F32, id16_in)
        id128 = cdma("id128s", [128, 128], F32R, id128_in)
        id128b = cdma("id128bs", [128, 128], BF16, id128b_in)
        md0 = cdma("md0s", [128, 256], BF16, md0_in)
        md1 = cdma("md1s", [128, 256], BF16, md1_in)
        bias_c = cdma("bias_cs", [128, 16], F32, bias_in)
        cosb = cdma("cosbs", [HD, TS], BF16, cos_in)
        sinb = cdma("sinbs", [HD, TS], BF16, sin_in)
        iota_sb = cdma("iota_sbs", [128, 16], I32, iota_in)
        esel = cdma("esels", [1, NE], F32, esel_in)
        triu_f = cdma("triu_fs", [128, 128], F32, triu_in)
        epsb = const.tile([128, 1], F32, name="epsb")
        nc.vector.memset(epsb[:], EPS)
        ones1_f = const.tile([1, 128], F32, name="ones1_f")
        nc.vector.memset(ones1_f[:], 1.0)
        onesp_f = const.tile([128, 1], F32, name="onesp_f")
        nc.vector.memset(onesp_f[:], 1.0)
        onesp_b = const.tile([128, 1], BF16, name="onesp_b")
        nc.vector.tensor_copy(onesp_b[:], onesp_f[:])

        def rms_norm(src_tiles, dst_pool, dst_name, dst_dt):
            out = []
            for j, xt in enumerate(src_tiles):
                scratch = np_pool.tile([128, HID], F32, name="nscratch",
                                       tag="nscratch")
                ssq = np_pool.tile([128, 1], F32, name="nssq", tag="nssq")
                nc.scalar.activation(
                    scratch[:], xt[:], mybir.ActivationFunctionType.Square,
                    accum_out=ssq[:])
                std = np_pool.tile([128, 1], F32, name="nstd", tag="nstd")
                nc.scalar.activation(
                    std[:], ssq[:], mybir.ActivationFunctionType.Sqrt,
                    bias=epsb[:], scale=1.0 / HID)
                rstd = np_pool.tile([128, 1], F32, name="nrstd", tag="nrstd")
                nc.vector.reciprocal(rstd[:], std[:])
                hn = dst_pool.tile([128, HID], dst_dt, name=f"{dst_name}_{j}")
                nc.vector.tensor_scalar_mul(hn[:], xt[:], rstd[:])
                out.append(hn)
            return out

        # ---- idx sentinel early (moe zeroing deferred to attention) ----
        with tc.tile_pool(name="zpool", bufs=1) as zpool:
            zidx = zpool.tile([128, 16], I32, name="zidx")
            nc.vector.memset(zidx[:], 4095)
            nc.gpsimd.dma_start(
                idx_buf[:].rearrange("(j p) one -> p (j one)", p=128),
                zidx[:])

        # ================= phase 1: norm1, X^T =================
        actx = ExitStack()  # pools that live through attention/o_proj
        qkT_pool = actx.enter_context(tc.tile_pool(name="qkT_pool", bufs=1))
        v_pool = actx.enter_context(tc.tile_pool(name="v_pool", bufs=1))
        att_pool = actx.enter_context(tc.tile_pool(name="att_pool", bufs=1))
        pvb_pool = actx.enter_context(tc.tile_pool(name="pvb_pool", bufs=1))

        kT = [None] * N_KV
        qT = [None] * N_HEADS
        v_tiles = []

        with tc.tile_pool(name="hn_pool", bufs=1) as hn_pool, \
             tc.tile_pool(name="xt_pool", bufs=1) as xt_pool, \
             tc.tile_pool(name="wv_pool", bufs=1) as wv_pool, \
             tc.tile_pool(name="wqk_pool", bufs=8) as wqk_pool, \
             tc.tile_pool(name="rope_pool", bufs=4) as rope_pool, \
             tc.tile_pool(name="ps_tp", bufs=2, space="PSUM") as ps_tp, \
             tc.tile_pool(name="ps_mm", bufs=4, space="PSUM") as ps_mm, \
             tc.tile_pool(name="psv", bufs=2, space="PSUM") as psv:
            # prefetch V weights on the gpsimd queue (used after K block)
            wv_tiles = []
            for k in range(16):
                wvt = wv_pool.tile([128, KVS], BF16, name=f"wv_{k}")
                nc.gpsimd.dma_start(wvt[:], wv_in[ts(k, 128), :])
                wv_tiles.append(wvt)

            hn_tiles = rms_norm(x_tiles, hn_pool, "hn", BF16)

            xT = []
            for k in range(16):
                xtile = xt_pool.tile([128, 256], BF16, name=f"xT_{k}")
                for j in range(2):
                    tp = ps_tp.tile([128, 128], BF16, name="tp_ps", tag="tp",
                                    space="PSUM")
                    nc.tensor.transpose(tp[:], hn_tiles[j][:, ts(k, 128)],
                                        id128b[:])
                    nc.vector.tensor_copy(xtile[:, ts(j, 128)], tp[:])
                xT.append(xtile)

            def rope(src):
                rot = rope_pool.tile([128, 256], BF16, name="rrot", tag="rot")
                nc.sync.dma_start(rot[0:H2, :], src[H2:HD, :])
                nc.sync.dma_start(rot[H2:HD, :], src[0:H2, :])
                ta = rope_pool.tile([128, 256], BF16, name="rta", tag="ra")
                tb = rope_pool.tile([128, 256], BF16, name="rtb", tag="rb")
                nc.vector.tensor_mul(ta[:], src[:], cosb[:])
                nc.vector.tensor_mul(tb[:], rot[:], sinb[:])
                return ta, tb

            def proj_block(ob, names):
                # one 512-col output block of wqkT -> 4 [128,256] bf16 tiles
                pss = [ps_mm.tile([128, 256], F32, name="qk_ps", tag="mm",
                                  space="PSUM") for _ in range(4)]
                for k in range(16):
                    q = nc.sync if (k % 2 == 0) else nc.scalar
                    wt = wqk_pool.tile([128, 512], BF16, name="wqk_t", tag="w")
                    q.dma_start(wt[:], wqk_in[ts(k, 128), ts(ob, 512)])
                    for oi in range(4):
                        nc.tensor.matmul(pss[oi][:], wt[:, ts(oi, 128)],
                                         xT[k][:], start=(k == 0),
                                         stop=(k == 15))
                outs = []
                for oi in range(4):
                    dst = qkT_pool.tile([128, 256], BF16, name=names[oi])
                    nc.vector.tensor_copy(dst[:], pss[oi][:])
                    ta, tb = rope(dst)
                    nc.vector.tensor_add(dst[:], ta[:], tb[:])
                    outs.append(dst)
                return outs

            # --- K first (output cols 2048..2560) ---
            kT[0:4] = proj_block(4, [f"kT_{i}" for i in range(4)])

            # --- V ---
            vps = [psv.tile([128, KVS], F32, name="v_ps", tag="v",
                            space="PSUM") for _ in range(2)]
            for k in range(16):
                for j in range(2):
                    nc.tensor.matmul(vps[j][:], xT[k][:, ts(j, 128)],
                                     wv_tiles[k][:], start=(k == 0),
                                     stop=(k == 15))
            for j in range(2):
                vt = v_pool.tile([128, KVS], BF16, name=f"v_{j}")
                nc.vector.tensor_copy(vt[:], vps[j][:])
                v_tiles.append(vt)

            # --- stage K/V and kick the KV AllGather ---
            for kv in range(N_KV):
                nc.sync.dma_start(
                    kv_ci[kv * 16:(kv + 1) * 16, :].rearrange(
                        "a (b t) -> (a b) t", t=TS),
                    kT[kv][:])
            for j in range(2):
                nc.sync.dma_start(
                    kv_ci[64 + 32 * j:64 + 32 * (j + 1), :].rearrange(
                        "a (b d) -> (a b) d", d=KVS),
                    v_tiles[j][:])
            nc.gpsimd.collective_compute(
                "AllGather", mybir.AluOpType.bypass, replica_groups=RG,
                ins=[kv_ci[:]], outs=[kv_co[:]])

            # --- Q (overlaps the AllGather) ---
            for ob in range(4):
                qT[4 * ob:4 * ob + 4] = proj_block(
                    ob, [f"qT_{4 * ob + i}" for i in range(4)])

            if KDBG:
                for o in range(16):
                    nc.sync.dma_start(dbg_q[ts(o, 128), :], qT[o][:])
                for kv in range(N_KV):
                    nc.sync.dma_start(dbg_k[ts(kv, 128), :], kT[kv][:])
                for j in range(2):
                    nc.sync.dma_start(dbg_v[ts(j, 128), :], v_tiles[j][:])

        # ============ pass B: diagonal attention with local KV ============
        pvB = [None] * N_HEADS
        rsB = [None] * N_HEADS
        with tc.tile_pool(name="eb_pool", bufs=4) as eb_pool, \
             tc.tile_pool(name="ps_bs", bufs=2, space="PSUM") as ps_bs, \
             tc.tile_pool(name="ps_bpv", bufs=2, space="PSUM") as ps_bpv, \
             tc.tile_pool(name="ps_brs", bufs=2, space="PSUM") as ps_brs:
            for kv in range(N_KV):
                for hp in range(2):
                    heads = [4 * kv + 2 * hp, 4 * kv + 2 * hp + 1]
                    pv_ps = ps_bpv.tile([128, 512], F32, name="bpv_ps",
                                        tag="pv", space="PSUM")
                    rs_ps = ps_brs.tile([1, 512], F32, name="brs_ps",
                                        tag="rs", space="PSUM")
                    for half in range(2):
                        sps = ps_bs.tile([128, 512], F32, name="bs_ps",
                                         tag="s", space="PSUM")
                        for i in range(2):
                            nc.tensor.matmul(sps[:, ts(i, 256)],
                                             kT[kv][:, ts(half, 128)],
                                             qT[heads[i]][:],
                                             start=True, stop=True)
                        et = eb_pool.tile([128, 512], BF16, name="bet",
                                          tag="e")
                        nc.scalar.activation(
                            et[:], sps[:],
                            mybir.ActivationFunctionType.Exp, scale=SCALE)
                        msk = md0 if half == 0 else md1
                        for i in range(2):
                            nc.vector.tensor_mul(et[:, ts(i, 256)],
                                                 et[:, ts(i, 256)], msk[:])
                        nc.tensor.matmul(pv_ps[:],
                                         v_tiles[half][:, ts(kv, 128)],
                                         et[:], start=(half == 0),
                                         stop=(half == 1))
                        nc.tensor.matmul(rs_ps[:], onesp_b[:], et[:],
                                         start=(half == 0),
                                         stop=(half == 1))
                    for i in range(2):
                        pb = pvb_pool.tile([128, 256], F32,
                                           name=f"pvB_{heads[i]}")
                        nc.vector.tensor_copy(pb[:], pv_ps[:, ts(i, 256)])
                        pvB[heads[i]] = pb
                        rb = pvb_pool.tile([1, 256], F32,
                                           name=f"rsB_{heads[i]}")
                        nc.vector.tensor_copy(rb[:], rs_ps[:, ts(i, 256)])
                        rsB[heads[i]] = rb
                        if KDBG:
                            nc.sync.dma_start(dbg_pvb[ts(heads[i], 128), :],
                                              pb[:])
                            nc.sync.dma_start(
                                dbg_rsb[heads[i]:heads[i] + 1, :], rb[:])

        # ================= pass A: gathered attention =================
        attnT = [None] * N_HEADS
        with tc.tile_pool(name="kvt_pool", bufs=1) as kvt_pool, \
             tc.tile_pool(name="e_pool", bufs=6) as e_pool, \
             tc.tile_pool(name="sc_pool", bufs=4) as sc_pool, \
             tc.tile_pool(name="ps_s", bufs=3, space="PSUM") as ps_s, \
             tc.tile_pool(name="ps_pv", bufs=2, space="PSUM") as ps_pv, \
             tc.tile_pool(name="ps_rs", bufs=2, space="PSUM") as ps_rs:
            katt = [[None] * N_KV for _ in range(NC)]
            vatt = [None] * 16
            qs = [nc.sync, nc.scalar, nc.gpsimd]
            qi = 0
            for r in range(NC):
                for kv in range(N_KV):
                    kt = kvt_pool.tile([128, 256], BF16, name=f"k_{r}_{kv}")
                    qs[qi % 3].dma_start(
                        kt[:],
                        kv_co[128 * r + 16 * kv:128 * r + 16 * (kv + 1), :]
                        .rearrange("a (b t) -> (a b) t", t=TS))
                    katt[r][kv] = kt
                    qi += 1
                for j in range(2):
                    vt = kvt_pool.tile([128, KVS], BF16, name=f"v_{r}_{j}")
                    qs[qi % 3].dma_start(
                        vt[:],
                        kv_co[128 * r + 64 + 32 * j:128 * r + 64 + 32 * (j + 1),
                              :].rearrange("a (b d) -> (a b) d", d=KVS))
                    vatt[2 * r + j] = vt
                    qi += 1

            # zero the moe scatter buffers (DMA engines idle during pass A)
            ztf = kvt_pool.tile([128, 512], F32, name="ztf")
            nc.vector.memset(ztf[:], 0.0)
            ztile = kvt_pool.tile([128, 512], BF16, name="ztile")
            nc.vector.tensor_copy(ztile[:], ztf[:])
            for q in range(4):
                for i in range(T // 128):
                    nc.gpsimd.dma_start(moe_q[q][ts(i, 128), :], ztile[:])

            if KDBG:
                for kv in range(N_KV):
                    nc.sync.dma_start(dbg_katt[ts(kv, 128), :],
                                      katt[0][kv][:])
                for j in range(2):
                    nc.sync.dma_start(dbg_vatt[ts(j, 128), :], vatt[j][:])
                kvraw = kvt_pool.tile([128, 2048], BF16, name="kvraw")
                nc.sync.dma_start(kvraw[:], kv_co[0:128, :])
                nc.sync.dma_start(dbg_kvco[:], kvraw[:])

            for kv in range(N_KV):
                for hp in range(2):
                    heads = [4 * kv + 2 * hp, 4 * kv + 2 * hp + 1]
                    pv_ps = ps_pv.tile([128, 512], F32, name="pv_ps",
                                       tag="pv", space="PSUM")
                    rs_ps = ps_rs.tile([1, 512], F32, name="rs_ps",
                                       tag="rs", space="PSUM")
                    for sg in range(16):
                        sps = ps_s.tile([128, 512], F32, name="s_ps", tag="s",
                                        space="PSUM")
                        for i in range(2):
                            nc.tensor.matmul(sps[:, ts(i, 256)],
                                             katt[sg // 2][kv][:,
                                                              ts(sg % 2, 128)],
                                             qT[heads[i]][:],
                                             start=True, stop=True)
                        et = e_pool.tile([128, 512], BF16, name="et", tag="e")
                        nc.scalar.activation(
                            et[:], sps[:],
                            mybir.ActivationFunctionType.Exp,
                            bias=bias_c[:, sg:sg + 1], scale=SCALE)
                        nc.tensor.matmul(pv_ps[:], vatt[sg][:, ts(kv, 128)],
                                         et[:], start=(sg == 0),
                                         stop=(sg == 15))
                        if sg % 2 == 0:
                            et_prev = et
                        else:
                            epair = e_pool.tile([128, 512], BF16,
                                                name="epair", tag="ep")
                            nc.vector.tensor_add(epair[:], et_prev[:], et[:])
                            nc.tensor.matmul(rs_ps[:], onesp_b[:], epair[:],
                                             start=(sg == 1),
                                             stop=(sg == 15))
                    for i in range(2):
                        h = heads[i]
                        rs_sb = sc_pool.tile([1, 256], F32R, name="rs_sb",
                                             tag="rsb")
                        nc.vector.tensor_add(rs_sb[:], rs_ps[:, ts(i, 256)],
                                             rsB[h][:])
                        with nc.allow_low_precision(
                                reason="f32r recip for PE bcast"):
                            nc.vector.reciprocal(rs_sb[:], rs_sb[:])
                        bc_ps = ps_s.tile([128, 256], F32, name="bc_ps",
                                          tag="s", space="PSUM")
                        nc.tensor.matmul(bc_ps[:], ones1_f[:].bitcast(F32R),
                                         rs_sb[:], start=True, stop=True)
                        bc_sb = sc_pool.tile([128, 256], F32, name="bc_sb",
                                             tag="bcs")
                        nc.scalar.copy(bc_sb[:], bc_ps[:])
                        pvt = sc_pool.tile([128, 256], F32, name="pv_tot",
                                           tag="pvt")
                        nc.vector.tensor_add(pvt[:], pv_ps[:, ts(i, 256)],
                                             pvB[h][:])
                        at = att_pool.tile([128, 256], BF16,
                                           name=f"attnT_{h}")
                        nc.vector.tensor_mul(at[:], pvt[:], bc_sb[:])
                        attnT[h] = at

        # ============ o_proj + residual ============
        resid2 = []
        with tc.tile_pool(name="wo_pool", bufs=5) as wo_pool, \
             tc.tile_pool(name="ps5", bufs=8, space="PSUM") as ps5:
            o_ps = [[ps5.tile([128, 512], F32, name="o_ps", tag="t",
                              space="PSUM") for _ in range(4)]
                    for _ in range(2)]
            for k in range(16):
                q = nc.sync if (k % 2 == 0) else nc.scalar
                wt = wo_pool.tile([128, HID], BF16, name="wo_t", tag="w")
                q.dma_start(wt[:], wo_in[ts(k, 128), :])
                for j in range(2):
                    for nb in range(4):
                        nc.tensor.matmul(o_ps[j][nb][:],
                                         attnT[k][:, ts(j, 128)],
                                         wt[:, ts(nb, 512)], start=(k == 0),
                                         stop=(k == 15))
            for j in range(2):
                r2 = r2_pool.tile([128, HID], F32, name=f"resid2_{j}")
                for nb in range(4):
                    nc.vector.tensor_add(r2[:, ts(nb, 512)], o_ps[j][nb][:],
                                         x_tiles[j][:, ts(nb, 512)])
                resid2.append(r2)
        if KDBG:
            for h in range(16):
                nc.sync.dma_start(dbg_att[ts(h, 128), :], attnT[h][:])
            for j in range(2):
                nc.sync.dma_start(dbg_r2[ts(j, 128), :], resid2[j][:])
        actx.close()

        # ============ norm2 + gate + fused h/w AllGather ============
        with tc.tile_pool(name="h2_pool", bufs=1) as h2_pool:
            h2n_tiles = rms_norm(resid2, h2_pool, "h2n", F32R)

            with tc.tile_pool(name="x2t_pool", bufs=1) as x2t_pool, \
                 tc.tile_pool(name="gate_pool", bufs=2) as gate_pool, \
                 tc.tile_pool(name="ps6t", bufs=2, space="PSUM") as ps6t, \
                 tc.tile_pool(name="ps6b", bufs=2, space="PSUM") as ps6b:
                x2T = []
                for k in range(16):
                    row = []
                    for j in range(2):
                        dst = x2t_pool.tile([128, 128], F32R,
                                            name=f"x2T_{k}_{j}")
                        tp = ps6t.tile([128, 128], F32R, name="tp2_ps",
                                       tag="t", space="PSUM")
                        nc.tensor.transpose(tp[:],
                                            h2n_tiles[j][:, ts(k, 128)],
                                            id128[:])
                        nc.vector.tensor_copy(dst[:], tp[:])
                        row.append(dst)
                    x2T.append(row)

                gsb = gate_pool.tile([128, 16 * NE], F32R, name="gsb")
                nc.sync.dma_start(
                    gsb[:].rearrange("p (k e) -> p k e", e=NE),
                    gate_in[:].rearrange("(k p) e -> p k e", p=128))
                for j in range(2):
                    gps = ps6b.tile([128, NE], F32, name="g_ps", tag="t",
                                    space="PSUM")
                    for k in range(16):
                        nc.tensor.matmul(
                            gps[:], x2T[k][j][:],
                            gsb[:].rearrange("p (k e) -> p k e", e=NE)[:, k, :],
                            start=(k == 0), stop=(k == 15))
                    lg = gate_pool.tile([128, NE], F32, name="lg", tag="g1")
                    nc.vector.tensor_copy(lg[:], gps[:])
                    mx = gate_pool.tile([128, 1], F32, name="gmx", tag="g2")
                    nc.vector.reduce_max(mx[:], lg[:],
                                         axis=mybir.AxisListType.X)
                    nmx = gate_pool.tile([128, 1], F32, name="gnmx", tag="g3")
                    nc.vector.tensor_scalar_mul(nmx[:], mx[:], -1.0)
                    p = gate_pool.tile([128, NE], F32, name="gp", tag="g4")
                    nc.scalar.activation(p[:], lg[:],
                                         mybir.ActivationFunctionType.Exp,
                                         bias=nmx[:])
                    v1 = gate_pool.tile([128, 1], F32, name="gv1", tag="g5")
                    nc.vector.reduce_max(v1[:], p[:],
                                         axis=mybir.AxisListType.X)
                    ge1 = gate_pool.tile([128, NE], F32, name="gge1", tag="g6")
                    nc.vector.tensor_single_scalar(ge1[:], p[:], v1[:],
                                                   op=mybir.AluOpType.is_ge)
                    pt = gate_pool.tile([128, NE], F32, name="gpt", tag="g7")
                    nc.vector.tensor_mul(pt[:], p[:], ge1[:])
                    p2 = gate_pool.tile([128, NE], F32, name="gp2", tag="g8")
                    nc.vector.tensor_sub(p2[:], p[:], pt[:])
                    v2 = gate_pool.tile([128, 1], F32, name="gv2", tag="g9")
                    nc.vector.reduce_max(v2[:], p2[:],
                                         axis=mybir.AxisListType.X)
                    m2 = gate_pool.tile([128, NE], F32, name="gm2", tag="g10")
                    nc.vector.tensor_single_scalar(m2[:], p[:], v2[:],
                                                   op=mybir.AluOpType.is_ge)
                    pm = gate_pool.tile([128, NE], F32, name="gpm", tag="g11")
                    nc.vector.tensor_mul(pm[:], p[:], m2[:])
                    s12 = gate_pool.tile([128, 1], F32, name="gs12", tag="g12")
                    nc.vector.tensor_add(s12[:], v1[:], v2[:])
                    nc.vector.reciprocal(s12[:], s12[:])
                    wful = h2_pool.tile([128, NE], F32, name=f"wful_{j}")
                    nc.vector.tensor_scalar_mul(wful[:], pm[:], s12[:])
                    nc.sync.dma_start(w_ci[ts(j, 128), :], wful[:])
                    if KDBG:
                        nc.sync.dma_start(dbg_w[ts(j, 128), :], wful[:])

                nc.gpsimd.collective_compute(
                    "AllGather", mybir.AluOpType.bypass, replica_groups=RG,
                    ins=[w_ci[:]], outs=[w_co[:]])

            # h cast + stage + AllGather (selection overlaps this)
            with tc.tile_pool(name="h2b_pool", bufs=2) as h2b_pool:
                for j in range(2):
                    hb = h2b_pool.tile([128, HID], BF16, name="h2b", tag="b")
                    nc.vector.tensor_copy(hb[:], h2n_tiles[j][:])
                    nc.sync.dma_start(h_ci[ts(j, 128), :], hb[:])
                    if KDBG:
                        nc.sync.dma_start(dbg_h2[ts(j, 128), :],
                                          h2n_tiles[j][:].bitcast(F32))
            nc.gpsimd.collective_compute(
                "AllGather", mybir.AluOpType.bypass, replica_groups=RG,
                ins=[h_ci[:]], outs=[h_co[:]])

        # ====== FFN weight prefetch (overlaps AllGather + selection) ======
        gat_pool = gctx.enter_context(tc.tile_pool(name="gat_pool", bufs=1))
        g_pool = gctx.enter_context(tc.tile_pool(name="g_pool", bufs=1))
        fctx = ExitStack()
        w13_pool = fctx.enter_context(tc.tile_pool(name="w13_pool", bufs=40))

        def load_w13(mb):
            w1ts, w3ts = [], []
            for k in range(16):
                w1t = w13_pool.tile([128, 512], BF16, name="w1_t", tag="w1")
                nc.sync.dma_start(w1t[:], w1_in[ts(k, 128), ts(mb, 512)])
                w1ts.append(w1t)
                w3t = w13_pool.tile([128, 512], BF16, name="w3_t", tag="w3")
                nc.scalar.dma_start(w3t[:], w3_in[ts(k, 128), ts(mb, 512)])
                w3ts.append(w3t)
            return w1ts, w3ts

        w13_pre = {mb: load_w13(mb) for mb in range(2)}

        # ================= expert token selection =================
        with tc.tile_pool(name="sel_pool", bufs=1) as sel_pool, \
             tc.tile_pool(name="ps7", bufs=2, space="PSUM") as ps7:
            wall = sel_pool.tile([128, 16 * NE], F32, name="wall")
            nc.sync.dma_start(
                wall[:].rearrange("p (k e) -> p k e", e=NE),
                w_co[:].rearrange("(k p) e -> p k e", p=128))
            eselb = sel_pool.tile([128, NE], F32, name="eselb")
            nc.gpsimd.partition_broadcast(eselb[:], esel[:])
            wsel = sel_pool.tile([128, 16 * NE], F32, name="wsel")
            nc.vector.tensor_tensor(
                wsel[:].rearrange("p (k e) -> p k e", e=NE),
                wall[:].rearrange("p (k e) -> p k e", e=NE),
                eselb[:].rearrange("p (o e) -> p o e", o=1)
                .to_broadcast([128, 16, NE]),
                op=mybir.AluOpType.mult)
            wcol = sel_pool.tile([128, 16], F32, name="wcol")
            nc.vector.reduce_sum(
                wcol[:], wsel[:].rearrange("p (k e) -> p k e", e=NE),
                axis=mybir.AxisListType.X)
            nc.sync.dma_start(
                wcol_d[:].rearrange("(j p) one -> p (j one)", p=128), wcol[:])
            mall = sel_pool.tile([128, 16], F32, name="mall")
            nc.vector.tensor_single_scalar(mall[:], wcol[:], 0.0,
                                           op=mybir.AluOpType.is_gt)
            rank_ps = ps7.tile([128, 16], F32, name="rank_ps", tag="a",
                               space="PSUM")
            nc.tensor.matmul(rank_ps[:], triu_f[:], mall[:], start=True,
                             stop=True)
            tot_ps = ps7.tile([1, 16], F32, name="tot_ps", tag="b",
                              space="PSUM")
            nc.tensor.matmul(tot_ps[:], onesp_f[:], mall[:], start=True,
                             stop=True)
            tot = sel_pool.tile([1, 16], F32, name="tot")
            nc.vector.tensor_copy(tot[:], tot_ps[:])
            totT_ps = ps7.tile([16, 1], F32, name="totT_ps", tag="b",
                               space="PSUM")
            nc.tensor.matmul(totT_ps[:], tot[:], ones1_f[:, 0:1], start=True,
                             stop=True)
            totT = sel_pool.tile([16, 1], F32, name="totT")
            nc.vector.tensor_copy(totT[:], totT_ps[:])
            ex_ps = ps7.tile([16, 1], F32, name="ex_ps", tag="b", space="PSUM")
            nc.tensor.matmul(ex_ps[:], su16[:], totT[:], start=True, stop=True)
            exT = sel_pool.tile([16, 1], F32, name="exT")
            nc.vector.tensor_copy(exT[:], ex_ps[:])
            exr_ps = ps7.tile([1, 16], F32, name="exr_ps", tag="b",
                              space="PSUM")
            nc.tensor.matmul(exr_ps[:], exT[:], id16[:], start=True, stop=True)
            exr = sel_pool.tile([1, 16], F32, name="exr")
            nc.vector.tensor_copy(exr[:], exr_ps[:])
            exb_ps = ps7.tile([128, 16], F32, name="exb_ps", tag="b",
                              space="PSUM")
            nc.tensor.matmul(exb_ps[:], ones1_f[:], exr[:], start=True,
                             stop=True)
            posf = sel_pool.tile([128, 16], F32, name="posf")
            nc.vector.tensor_copy(posf[:], rank_ps[:])
            nc.vector.tensor_add(posf[:], posf[:], exb_ps[:])
            adj = sel_pool.tile([128, 16], F32, name="adj")
            nc.vector.tensor_scalar(
                adj[:], mall[:], -4096.0, 4095.0,
                op0=mybir.AluOpType.mult, op1=mybir.AluOpType.add)
            nc.vector.tensor_add(posf[:], posf[:], adj[:])
            posi = sel_pool.tile([128, 16], I32, name="posi")
            nc.vector.tensor_copy(posi[:], posf[:])
            for j in range(16):
                nc.gpsimd.indirect_dma_start(
                    out=idx_buf[:],
                    out_offset=IndirectOffsetOnAxis(ap=posi[:, j:j + 1],
                                                    axis=0),
                    in_=iota_sb[:, j:j + 1],
                    in_offset=None,
                    bounds_check=T - 1, oob_is_err=False)

        idx_all = gat_pool.tile([128, NG], I32, name="idx_all")
        nc.sync.dma_start(
            idx_all[:],
            idx_buf[0:NG * 128, :].rearrange("(g p) one -> p (g one)", p=128))
        idx_tiles, wg_tiles = [], []
        for g in range(NG):
            it = idx_all[:, g:g + 1]
            idx_tiles.append(it)
            wg = gat_pool.tile([128, 1], F32, name=f"wg_{g}")
            nc.vector.memset(wg[:], 0.0)
            nc.gpsimd.indirect_dma_start(
                out=wg[:], out_offset=None,
                in_=wcol_d[:],
                in_offset=IndirectOffsetOnAxis(ap=it, axis=0),
                bounds_check=T - 1, oob_is_err=False)
            wg_tiles.append(wg)
            if KDBG:
                nc.sync.dma_start(dbg_idx[ts(g, 128), :], it)
                nc.sync.dma_start(dbg_wg[ts(g, 128), :], wg[:])

        # ================= gather + expert FFN =================
        g_tiles = []

        with tc.tile_pool(name="xgt_pool", bufs=1) as xgt_pool:
            xgT = [xgt_pool.tile([128, CAP], BF16, name=f"xgT_{k}")
                   for k in range(16)]
            with tc.tile_pool(name="row_pool", bufs=2) as row_pool, \
                 tc.tile_pool(name="ps8", bufs=3, space="PSUM") as ps8:
                for g in range(NG):
                    rows = row_pool.tile([128, HID], BF16, name="xg_rows",
                                         tag="rows")
                    nc.gpsimd.indirect_dma_start(
                        out=rows[:], out_offset=None,
                        in_=h_co[:],
                        in_offset=IndirectOffsetOnAxis(
                            ap=idx_tiles[g], axis=0),
                        bounds_check=T - 1, oob_is_err=False)
                    for k in range(16):
                        tp = ps8.tile([128, 128], BF16, name="tg_ps", tag="t",
                                      space="PSUM")
                        nc.tensor.transpose(tp[:], rows[:, ts(k, 128)],
                                            id128b[:])
                        nc.vector.tensor_copy(xgT[k][:, ts(g, 128)],
                                              tp[:])
                if KDBG:
                    for k in range(16):
                        nc.sync.dma_start(dbg_xg[ts(k, 128), :], xgT[k][:])

            with tc.tile_pool(name="silu_pool", bufs=3) as silu_pool, \
                 tc.tile_pool(name="ps_f", bufs=8, space="PSUM") as ps_f:
                for mb in range(8):
                    w1ts, w3ts = w13_pre.pop(mb)
                    if mb + 2 < 8:
                        w13_pre[mb + 2] = load_w13(mb + 2)
                    for mi in range(4):
                        m = 4 * mb + mi
                        h1_ps = [ps_f.tile([128, NW], F32, name="h1_ps",
                                           tag="t", space="PSUM")
                                 for _ in range(NSPL)]
                        h3_ps = [ps_f.tile([128, NW], F32, name="h3_ps",
                                           tag="t", space="PSUM")
                                 for _ in range(NSPL)]
                        for k in range(16):
                            for s in range(NSPL):
                                nc.tensor.matmul(h1_ps[s][:],
                                                 w1ts[k][:, ts(mi, 128)],
                                                 xgT[k][:, ts(s, NW)],
                                                 start=(k == 0),
                                                 stop=(k == 15))
                            for s in range(NSPL):
                                nc.tensor.matmul(h3_ps[s][:],
                                                 w3ts[k][:, ts(mi, 128)],
                                                 xgT[k][:, ts(s, NW)],
                                                 start=(k == 0),
                                                 stop=(k == 15))
                        gt = g_pool.tile([128, CAP], BF16, name=f"g_{m}")
                        for s in range(NSPL):
                            s1 = silu_pool.tile([128, NW], BF16,
                                                name="silu_t", tag="s")
                            nc.scalar.activation(
                                s1[:], h1_ps[s][:],
                                mybir.ActivationFunctionType.Silu)
                            nc.vector.tensor_mul(gt[:, ts(s, NW)], s1[:],
                                                 h3_ps[s][:])
                        g_tiles.append(gt)
                        if KDBG:
                            nc.sync.dma_start(dbg_g[ts(m, 128), :], gt[:])
        fctx.close()

        # w2 + transpose back + per-chunk scale/scatter + 4-chunk RS
        with tc.tile_pool(name="orow_pool", bufs=1) as orow_pool, \
             tc.tile_pool(name="oe_pool", bufs=2) as oe_pool, \
             tc.tile_pool(name="w2_pool", bufs=56) as w2_pool, \
             tc.tile_pool(name="ps_w", bufs=4, space="PSUM") as ps_w, \
             tc.tile_pool(name="ps_wt", bufs=3, space="PSUM") as ps_wt:
            orows = [orow_pool.tile([128, HID], BF16, name=f"orow_{g}")
                     for g in range(NG)]

            for db in range(4):
                w2ts = []
                for m in range(32):
                    q = nc.sync if (m % 2 == 0) else nc.scalar
                    w2t = w2_pool.tile([128, 512], BF16, name="w2_t", tag="w")
                    q.dma_start(w2t[:], w2_in[ts(m, 128), ts(db, 512)])
                    w2ts.append(w2t)
                for di in range(4):
                    d = 4 * db + di
                    o_ps = [ps_w.tile([128, NW], F32, name="oe_ps", tag="t",
                                      space="PSUM") for _ in range(NSPL)]
                    for m in range(32):
                        for s in range(NSPL):
                            nc.tensor.matmul(o_ps[s][:],
                                             w2ts[m][:, ts(di, 128)],
                                             g_tiles[m][:, ts(s, NW)],
                                             start=(m == 0), stop=(m == 31))
                    oe = oe_pool.tile([128, CAP], BF16, name="oe", tag="oe")
                    for s in range(NSPL):
                        nc.vector.tensor_copy(oe[:, ts(s, NW)], o_ps[s][:])
                    for g in range(NG):
                        tp = ps_wt.tile([128, 128], BF16, name="to_ps",
                                        tag="t", space="PSUM")
                        nc.tensor.transpose(tp[:], oe[:, ts(g, 128)],
                                            id128b[:])
                        nc.vector.tensor_copy(orows[g][:, ts(d, 128)], tp[:])
                # this 512-col chunk of all orows is complete: scale,
                # scatter into its own buffer, and reduce-scatter it
                c0 = db * 512
                for g in range(NG):
                    nc.vector.tensor_scalar_mul(
                        orows[g][:, c0:c0 + 512],
                        orows[g][:, c0:c0 + 512], wg_tiles[g][:])
                    nc.gpsimd.indirect_dma_start(
                        out=moe_q[db][:],
                        out_offset=IndirectOffsetOnAxis(
                            ap=idx_tiles[g], axis=0),
                        in_=orows[g][:, c0:c0 + 512],
                        in_offset=None,
                        bounds_check=T - 1, oob_is_err=False)
                nc.gpsimd.collective_compute(
                    "ReduceScatter", mybir.AluOpType.add, replica_groups=RG,
                    ins=[moe_q[db][:]], outs=[rs_q[db][:]])
            if KDBG:
                for g in range(NG):
                    nc.sync.dma_start(dbg_or[ts(g, 128), :], orows[g][:])

        # ================= final residual add =================
        with tc.tile_pool(name="fin_pool", bufs=4) as fin_pool:
            for db in range(4):
                for j in range(2):
                    rt = fin_pool.tile([128, 512], BF16, name="rs_t",
                                       tag="r")
                    nc.sync.dma_start(rt[:], rs_q[db][ts(j, 128), :])
                    ft = fin_pool.tile([128, 512], F32, name="fin_t",
                                       tag="f")
                    nc.vector.tensor_add(
                        ft[:], rt[:],
                        resid2[j][:, db * 512:(db + 1) * 512])
                    nc.sync.dma_start(
                        y_out[ts(j, 128), db * 512:(db + 1) * 512], ft[:])

    nc.finalize()
    return nc


def _host_inputs(hidden, positions, norm1_w, norm2_w, wqkv, wo, gate_w, w1, w2,
                 w3):
    f = np.float32
    bf = ml_dtypes.bfloat16
    hidden = np.asarray(hidden, f)
    positions = np.asarray(positions, np.int32)
    norm1_w = np.asarray(norm1_w, f)
    norm2_w = np.asarray(norm2_w, f)
    wqkv = np.asarray(wqkv, f)
    wo = np.asarray(wo, f)
    gate_w = np.asarray(gate_w, f)
    w1 = np.asarray(w1, f)
    w2 = np.asarray(w2, f)
    w3 = np.asarray(w3, f)

    wqkvT = (wqkv * norm1_w[None, :]).T.copy()
    wqkT = np.ascontiguousarray(wqkvT[:, : QS + KVS]).astype(bf)
    wvT = np.ascontiguousarray(wqkvT[:, QS + KVS:]).astype(bf)
    woT = np.ascontiguousarray(wo.T).astype(bf)
    gateT = np.ascontiguousarray((gate_w * norm2_w[None, :]).T)

    half = HD // 2
    inv_freq = 1.0 / (ROPE_THETA ** (np.arange(0, half, dtype=f) * 2.0 / HD))
    ang = positions.astype(f)[:, None] * inv_freq[None, :]
    c = np.cos(ang).T.astype(f)  # [half, T]
    s = np.sin(ang).T.astype(f)
    cosT = np.concatenate([c, c], axis=0).astype(bf)  # [HD, T]
    sinT = np.concatenate([-s, s], axis=0).astype(bf)  # rotate-half sign

    triu128 = np.triu(np.ones((128, 128), f))
    su16 = np.triu(np.ones((16, 16), f), k=1)
    id16 = np.eye(16, dtype=f)
    id128 = np.eye(128, dtype=f)
    id128b = np.eye(128, dtype=f).astype(bf)
    md0 = np.concatenate([triu128, np.ones((128, 128), f)], axis=1).astype(bf)
    md1 = np.concatenate([np.zeros((128, 128), f), triu128],
                         axis=1).astype(bf)
    iota_c = (np.arange(16)[None, :] * 128
              + np.arange(128)[:, None]).astype(np.int32)

    in_maps = []
    for c_ in range(NC):
        sl = slice(c_ * TS, (c_ + 1) * TS)
        bias_c = np.zeros((128, 16), f)
        bias_c[:, 2 * c_:] = NEG  # diagonal + future blocks excluded in pass A
        e_sel = np.zeros((1, NE), f)
        e_sel[0, c_] = 1.0
        in_maps.append({
            "x": np.ascontiguousarray(hidden[sl]),
            "cos_t": np.ascontiguousarray(cosT[:, sl]),
            "sin_t": np.ascontiguousarray(sinT[:, sl]),
            "wqkT": wqkT,
            "wvT": wvT,
            "woT": woT,
            "gateT": gateT,
            "w1T": np.ascontiguousarray(
                (w1[c_] * norm2_w[None, :]).T.astype(bf)),
            "w3T": np.ascontiguousarray(
                (w3[c_] * norm2_w[None, :]).T.astype(bf)),
            "w2T": np.ascontiguousarray(w2[c_].T.astype(bf)),
            "triu128": triu128,
            "su16": su16,
            "id16": id16,
            "id128": id128,
            "id128b": id128b,
            "md0": md0,
            "md1": md1,
            "bias_c": bias_c,
            "e_sel": e_sel,
            "iota_c": iota_c,
        })
    return in_maps


def kernel(hidden_states, positions, norm1_w, norm2_w, wqkv, wo, gate_w, w1,
           w2, w3, _trace=False):
    if "nc" not in _cache:
        _cache["nc"] = build()
    nc = _cache["nc"]
    in_maps = _host_inputs(
        hidden_states, positions, norm1_w, norm2_w, wqkv, wo, gate_w, w1, w2,
        w3)
    res = run_bass_kernel_spmd(nc, in_maps, core_ids=list(range(NC)),
                               trace=_trace)
    _cache["last_result"] = res
    out = np.concatenate([res.results[c]["y"] for c in range(NC)], axis=0)
    return out.astype(np.float32)
